# revision 1
# baseline (speedup 1.0000x reference)
"""BitNet DiT on 8 Trainium2 NeuronCores — data-parallel over batch (2 images/core).

Host: patchify, time-embedding + adaLN modulation vectors, BitNet weight
quantization (ternary * per-tensor scale) -> bf16 upload.
Device: full 12-block DiT forward per core in a single Bass/Tile kernel.
BitNet matmuls run as exact integer arithmetic in bf16 (|values| <= 127,
fp32 accumulate). Attention runs in fp32 via transposed-logits + ones-column
softmax-denominator trick.

v2: activation transposes ride the DMA xbar (dma_start_transpose) instead of
PE+DVE; fused scalar_tensor_tensor for modulation and evac+residual;
in-place magic rounding; rstd via Sqrt+reciprocal (no ACT table thrash);
software-pipelined attention heads.
"""
import math
import os
import sys
import numpy as np

sys.path.insert(0, "/opt/trn_rl_repo")

import ml_dtypes  # noqa: E402
import concourse.bass as bass  # noqa: E402
import concourse.mybir as mybir  # noqa: E402
import concourse.tile as tile  # noqa: E402
from concourse import bacc  # noqa: E402
from concourse.bass_utils import run_bass_kernel_spmd  # noqa: E402
from concourse.masks import make_identity  # noqa: E402

F32 = mybir.dt.float32
F32R = mybir.dt.float32r
FP8 = mybir.dt.float8e4
BF16 = mybir.dt.bfloat16
AX = mybir.AxisListType
OP = mybir.AluOpType
AF = mybir.ActivationFunctionType

DIM = 768
DEPTH = int(os.environ.get("KERNEL_DEPTH", "12"))
HEADS = 12
HD = 64
PATCH = 16
IMG = 256
CIN = 3
HID = 4 * DIM
EPS = 1e-6
P = 128
T = 512            # tokens per core (2 images x 256)
NT = T // P        # 4 token tiles
NTOK = 256         # tokens per image
KD = DIM // P      # 6
KH = HID // P      # 24
MAGIC = float(np.float32(3 * 2**22))  # 12582912.0 RNE round-to-int magic

_CACHED = {}


def _mm_chunks(n):
    out = []
    s = 0
    while s < n:
        e = min(s + 512, n)
        out.append((s, e))
        s = e
    return out


def build_program(depth=DEPTH):
    nc = bacc.Bacc("TRN2", target_bir_lowering=False, debug=False, num_devices=8)

    xpT_d = nc.declare_dram_parameter("xpT", [DIM, T], F32, isOutput=False)
    posb_d = nc.declare_dram_parameter("posb", [NTOK, DIM], F32, isOutput=False)
    patchWT_d = nc.declare_dram_parameter("patchWT", [DIM, DIM], F32, isOutput=False)
    headWT_d = nc.declare_dram_parameter("headWT", [DIM, DIM], F32, isOutput=False)
    headb_d = nc.declare_dram_parameter("headb", [1, DIM], F32, isOutput=False)
    wqkv_d = nc.declare_dram_parameter("wqkv", [depth, DIM, 3 * DIM], FP8, isOutput=False)
    wproj_d = nc.declare_dram_parameter("wproj", [depth, DIM, DIM], FP8, isOutput=False)
    wfc1_d = nc.declare_dram_parameter("wfc1", [depth, DIM, HID], FP8, isOutput=False)
    wfc2_d = nc.declare_dram_parameter("wfc2", [depth, HID, DIM], FP8, isOutput=False)
    # modulation vectors: [block, norm(2), part, img(2), A/B(2), 768]
    mods_d = nc.declare_dram_parameter("mods", [depth, 2, P, 2, 2, DIM], F32, isOutput=False)
    wscl_d = nc.declare_dram_parameter("wscl", [1, 4 * depth], F32, isOutput=False)
    out_d = nc.declare_dram_parameter("zout", [T, DIM], F32, isOutput=True)

    with tile.TileContext(nc) as tc:
        from contextlib import ExitStack
        with ExitStack() as _ctx:
            constp = _ctx.enter_context(tc.tile_pool(name="const", bufs=1))
            residp = _ctx.enter_context(tc.tile_pool(name="resid", bufs=1))
            fm6p = _ctx.enter_context(tc.tile_pool(name="fm6", bufs=2))
            wp = _ctx.enter_context(tc.tile_pool(name="w", bufs=6))
            modp = _ctx.enter_context(tc.tile_pool(name="mod", bufs=2))
            tmp_ = _ctx.enter_context(tc.tile_pool(name="tm", bufs=1))
            gp = _ctx.enter_context(tc.tile_pool(name="g", bufs=4))
            hp = _ctx.enter_context(tc.tile_pool(name="h", bufs=5))
            xqp6 = _ctx.enter_context(tc.tile_pool(name="xqp6", bufs=3))
            xqp24 = _ctx.enter_context(tc.tile_pool(name="xqp24", bufs=2))
            xq6p = _ctx.enter_context(tc.tile_pool(name="xq6", bufs=7))
            xq24p = _ctx.enter_context(tc.tile_pool(name="xq24", bufs=2))
            eTp = _ctx.enter_context(tc.tile_pool(name="eT", bufs=2))
            scp = _ctx.enter_context(tc.tile_pool(name="sc", bufs=64))
            ps_mm = _ctx.enter_context(tc.tile_pool(name="ps_mm", bufs=2, space="PSUM"))
            ps_tp = _ctx.enter_context(tc.tile_pool(name="ps_tp", bufs=2, space="PSUM"))
            ps_lt = _ctx.enter_context(tc.tile_pool(name="ps_lt", bufs=2, space="PSUM"))
            ps_oa = _ctx.enter_context(tc.tile_pool(name="ps_oa", bufs=2, space="PSUM"))

            idf = constp.tile([P, P], F32)
            make_identity(nc, idf[:])

            # broadcast w_scales/127 to all partitions
            wsrow = constp.tile([1, 4 * depth], F32)
            nc.sync.dma_start(wsrow[:], wscl_d[:])
            wsb = constp.tile([P, 4 * depth], F32)
            nc.gpsimd.partition_broadcast(wsb[:], wsrow[0:1, :])
            nmag = constp.tile([P, 1], F32)
            nc.vector.memset(nmag[:], -MAGIC)
            pmag = constp.tile([P, 1], F32)
            nc.vector.memset(pmag[:], MAGIC)

            z = residp.tile([P, NT, DIM], F32)
            v_aug = residp.tile([P, NT, HEADS, HD + 1], F32)
            nc.vector.memset(v_aug[:, :, :, HD], 1.0)
            o_tm = residp.tile([P, NT, DIM], F32)

            # ---------------- patch embed ----------------
            posb_sb = gp.tile([P, 2, DIM], F32, tag="g", name="posb_sb")
            nc.sync.dma_start(posb_sb[:], posb_d.rearrange("(a p) d -> p a d", p=P))
            xpT = fm6p.tile([P, KD, T], F32, tag="fm6")
            nc.sync.dma_start(xpT[:], xpT_d.rearrange("(o p) t -> p o t", p=P))
            pw_pieces = []
            for i in range(3):
                pwp = gp.tile([P, 2, DIM], F32, tag="g", name="pwp")
                nc.gpsimd.dma_start(
                    pwp[:], patchWT_d[i * 2 * P:(i + 1) * 2 * P, :].rearrange(
                        "(o p) d -> p o d", p=P))
                pw_pieces.append(pwp)
            for t in range(NT):
                for (cs, ce) in _mm_chunks(DIM):
                    pt = ps_mm.tile([P, 512], F32, tag="mm", name="pmm")[:, : ce - cs]
                    for k in range(KD):
                        nc.tensor.matmul(pt[:], xpT[:, k, t * P:(t + 1) * P],
                                         pw_pieces[k // 2][:, k % 2, cs:ce],
                                         start=(k == 0), stop=(k == KD - 1))
                    nc.vector.tensor_tensor(z[:, t, cs:ce], pt[:], posb_sb[:, t % 2, cs:ce], OP.add)

            def load_w(dram, b, kchunks, width, npieces):
                """Stage one linear's transposed fp8 weights as npieces tiles."""
                span = kchunks // npieces
                tiles = []
                for i in range(npieces):
                    wt = wp.tile([P, span, width], FP8, tag="w")
                    nc.gpsimd.dma_start(
                        wt[:],
                        dram[b, i * span * P:(i + 1) * span * P, :].rearrange(
                            "(o p) f -> p o f", p=P))
                    tiles.append(wt)
                return tiles, span

            def quant_smalls(src_ap, ws_idx):
                """Per-token quant scales from one [128,w] source.
                Returns (s127, c)."""
                amax = scp.tile([P, 1], F32, tag="sc", name="amax")
                nc.vector.tensor_reduce(amax[:], src_ap, axis=AX.X, op=OP.max,
                                        apply_absolute_value=True)
                ac = scp.tile([P, 1], F32, tag="sc", name="amaxc")
                nc.vector.tensor_scalar_max(ac[:], amax[:], 1e-5)
                rs = scp.tile([P, 1], F32, tag="sc", name="rcp")
                nc.vector.reciprocal(rs[:], ac[:])
                s127 = scp.tile([P, 1], F32, tag="sc", name="s127")
                nc.vector.tensor_scalar_mul(s127[:], rs[:], 127.0)
                c = scp.tile([P, 1], F32, tag="sc", name="cc")
                nc.vector.tensor_scalar(c[:], ac[:], wsb[:, ws_idx:ws_idx + 1],
                                        None, OP.mult)
                return s127, c

            def quant_round_dma(src_ap, kchunks, s127):
                """In-place magic-round src*(s127) on ACT, unmagic to bf16 (ACT),
                DMA-transpose. Returns the transposed xqT tile [P, KD, 128]."""
                nc.scalar.activation(src_ap, src_ap, AF.Identity, scale=s127[:],
                                     bias=pmag[:])
                xq = xqp6.tile([P, DIM], BF16, tag="xqp6", name="xq6s")
                dst = xq6p.tile([P, KD, P], BF16, tag="xq6")
                nc.scalar.activation(xq[:], src_ap, AF.Identity, bias=nmag[:])
                nc.sync.dma_start_transpose(dst[:], xq[:])
                return dst

            def quant_round_dma_hid(ghalves, s127):
                """HID version: two [P,1536] halves, unmagic on DVE."""
                dst = xq24p.tile([P, KH, P], BF16, tag="xq24")
                for i, gh in enumerate(ghalves):
                    nc.scalar.activation(gh[:], gh[:], AF.Identity, scale=s127[:],
                                         bias=pmag[:])
                    xq = xqp24.tile([P, HID // 2], BF16, tag="xqp24", name="xq24s")
                    nc.vector.tensor_scalar(xq[:], gh[:], MAGIC, None, OP.subtract)
                    nc.sync.dma_start_transpose(dst[:, i * 12:(i + 1) * 12, :], xq[:])
                return dst

            def rstd_from_ssq(ssq):
                ms = scp.tile([P, 1], F32, tag="sc", name="msn")
                nc.vector.tensor_scalar(ms[:], ssq[:], 1.0 / DIM, EPS, OP.mult, OP.add)
                srt = scp.tile([P, 1], F32, tag="sc", name="srt")
                nc.scalar.activation(srt[:], ms[:], AF.Sqrt)
                rst = scp.tile([P, 1], F32, tag="sc", name="rstn")
                nc.vector.reciprocal(rst[:], srt[:])
                return rst

            def norm_mod(t, mt, rstd, dst):
                """dst = (z[t]*rstd) * modA + modB  (2 DVE ops via STT)."""
                img = t // 2
                nc.vector.scalar_tensor_tensor(dst, z[:, t, :], rstd[:],
                                               mt[:, img, 0, :], OP.mult, OP.mult)
                nc.gpsimd.tensor_tensor(dst, dst, mt[:, img, 1, :], OP.add)

            # ---- prologue: phase 1 of block 0 ----
            def load_mods(b_, n_, name):
                mt = modp.tile([P, 2, 2, DIM], F32, tag="mod", name=name)
                nc.gpsimd.dma_start(mt[:], mods_d[b_, n_])
                return mt

            mt1_nxt = load_mods(0, 0, "mt1")
            xq1_cur = [None] * NT
            cq8_cur = [None] * NT
            c_cur = [None] * NT
            ssq0 = [None] * NT
            sq_scr = tmp_.tile([P, DIM], F32, tag="tm", name="sqscr")
            for t in range(NT):
                sv = scp.tile([P, 1], F32, tag="sc", name="ssq")
                nc.scalar.activation(sq_scr[:], z[:, t, :], AF.Square, accum_out=sv[:])
                ssq0[t] = sv
            for t in range(NT):
                rst = rstd_from_ssq(ssq0[t])
                h = hp.tile([P, DIM], F32, tag="h")
                norm_mod(t, mt1_nxt, rst, h[:])
                s127, c = quant_smalls(h[:], 0)
                c_cur[t] = c
                cq8 = scp.tile([P, 1], F32, tag="sc", name="cq8")
                nc.vector.tensor_scalar_mul(cq8[:], c[:], 0.125)
                cq8_cur[t] = cq8
                xq1_cur[t] = quant_round_dma(h[:], KD, s127)

            for b in range(depth):
                xq1s, cq8s, c_list = xq1_cur, cq8_cur, c_cur
                mt2 = load_mods(b, 1, "mt2")

                wq_tiles, wq_half = load_w(wqkv_d, b, KD, 3 * DIM, 2)
                q_fm = fm6p.tile([P, KD, T], F32R, tag="fm6")
                k_fm = fm6p.tile([P, KD, T], F32R, tag="fm6")

                # --- phase 2: qkv + q/k transposes (pipelined by one tile) ---
                q_tms = [None] * NT
                k_tms = [None] * NT

                def p2_mm(t):
                    q_tm = hp.tile([P, DIM], F32, tag="h", name="q_tm")
                    k_tm = hp.tile([P, DIM], F32, tag="h", name="k_tm")
                    q_tms[t], k_tms[t] = q_tm, k_tm
                    for (cs, ce) in _mm_chunks(3 * DIM):
                        pt = ps_mm.tile([P, 512], F32, tag="mm", name="pmm")[:, : ce - cs]
                        for k in range(KD):
                            wt = wq_tiles[k // wq_half]
                            nc.tensor.matmul(pt[:], xq1s[t][:, k, :],
                                             wt[:, k % wq_half, cs:ce],
                                             start=(k == 0), stop=(k == KD - 1))
                        segs = []
                        if cs < DIM:
                            segs.append((cs, min(ce, DIM), "q"))
                        if ce > DIM and cs < 2 * DIM:
                            segs.append((max(cs, DIM), min(ce, 2 * DIM), "k"))
                        if ce > 2 * DIM:
                            segs.append((max(cs, 2 * DIM), ce, "v"))
                        for (s0, s1, kind) in segs:
                            po = pt[:, s0 - cs:s1 - cs]
                            if kind == "q":
                                nc.scalar.activation(q_tm[:, s0:s1], po, AF.Identity,
                                                     scale=cq8s[t][:])
                            elif kind == "k":
                                nc.scalar.activation(k_tm[:, s0 - DIM:s1 - DIM], po,
                                                     AF.Identity, scale=c_list[t][:])
                            else:
                                h0 = (s0 - 2 * DIM) // HD
                                h1 = (s1 - 2 * DIM) // HD
                                nc.scalar.activation(
                                    v_aug[:, t, h0:h1, 0:HD], po, AF.Identity,
                                    scale=c_list[t][:])

                def p2_tp(t):
                    # 12 PE transposes batched 4-per-PSUM-bank, 3 DVE copies each dst
                    for half, src in ((0, q_tms[t]), (1, k_tms[t])):
                        fm = q_fm if half == 0 else k_fm
                        for g0 in range(0, KD, 4):
                            gn = min(4, KD - g0)
                            ptb = ps_tp.tile([P, 512], F32, tag="tp", name="ptb")[:, : gn * P]
                            for j in range(gn):
                                nc.tensor.transpose(ptb[:, j * P:(j + 1) * P],
                                                    src[:, (g0 + j) * P:(g0 + j + 1) * P],
                                                    idf[:])
                            nc.vector.tensor_copy(
                                fm[:, g0:g0 + gn, t * P:(t + 1) * P], ptb[:])

                with nc.named_scope(f"b{b}_qkv"):
                    for t in range(NT):
                        p2_mm(t)
                        if t >= 1:
                            p2_tp(t - 1)
                    p2_tp(NT - 1)

                # --- phase 3: attention, heads pipelined by one ---
                wp_tiles, wp_half = load_w(wproj_d, b, KD, DIM, 2)
                pairs = [(img, hh) for img in range(2) for hh in range(HEADS)]
                eTs = {}

                def attn_lt(img, hh):
                    po = (hh % 2) * HD
                    ch = hh // 2
                    lt = ps_lt.tile([P, 2, NTOK], F32, tag="lt")
                    for mt in range(2):
                        nc.tensor.matmul(
                            lt[:, mt, :],
                            k_fm[po:po + HD, ch, img * NTOK + mt * P: img * NTOK + (mt + 1) * P],
                            q_fm[po:po + HD, ch, img * NTOK: (img + 1) * NTOK],
                            start=True, stop=True)
                    eT = eTp.tile([P, 2, NTOK], F32, tag="eT")
                    nc.scalar.activation(eT[:], lt[:], AF.Exp)
                    eTs[(img, hh)] = eT

                def attn_oa(img, hh):
                    eT = eTs.pop((img, hh))
                    for nt in range(2):
                        oa = ps_oa.tile([P, HD + 1], F32, tag="oa")
                        for mt in range(2):
                            nc.tensor.matmul(
                                oa[:], eT[:, mt, nt * P:(nt + 1) * P],
                                v_aug[:, img * 2 + mt, hh, :],
                                start=(mt == 0), stop=(mt == 1))
                        rinv = scp.tile([P, 1], F32, tag="sc")
                        nc.vector.reciprocal(rinv[:], oa[:, HD:HD + 1])
                        nc.scalar.activation(
                            o_tm[:, img * 2 + nt, hh * HD:(hh + 1) * HD],
                            oa[:, 0:HD], AF.Identity, scale=rinv[:])

                with nc.named_scope(f"b{b}_attn"):
                    for i in range(len(pairs) + 1):
                        if i < len(pairs):
                            attn_lt(*pairs[i])
                        if i > 0:
                            attn_oa(*pairs[i - 1])

                # --- o-quant + proj + n2 chain (pipelined) ---
                xqo = [None] * NT
                cps = [None] * NT

                def o_quant(t):
                    s127, c = quant_smalls(o_tm[:, t, :], 4 * b + 1)
                    cps[t] = c
                    xqo[t] = quant_round_dma(o_tm[:, t, :], KD, s127)

                with nc.named_scope(f"b{b}_oq"):
                    for t in range(NT):
                        o_quant(t)

                wf1_tiles, wf1_half = load_w(wfc1_d, b, KD, HID, 3)
                xq2 = [None] * NT
                c3s = [None] * NT
                ssq2 = [None] * NT
                s1272 = [None] * NT
                h2s = [None] * NT

                def n2a(t):
                    sq = tmp_.tile([P, DIM], F32, tag="tm", name="sqn")
                    sv = scp.tile([P, 1], F32, tag="sc", name="ssqn")
                    nc.scalar.activation(sq[:], z[:, t, :], AF.Square, accum_out=sv[:])
                    ssq2[t] = sv

                def n2b(t):
                    rst = rstd_from_ssq(ssq2[t])
                    h = hp.tile([P, DIM], F32, tag="h")
                    h2s[t] = h
                    norm_mod(t, mt2, rst, h[:])
                    s127, c = quant_smalls(h[:], 4 * b + 2)
                    c3s[t] = c
                    s1272[t] = s127

                def n2c(t):
                    xq2[t] = quant_round_dma(h2s[t][:], KD, s1272[t])

                with nc.named_scope(f"b{b}_proj"):
                    for t in range(NT):
                        for (cs, ce) in _mm_chunks(DIM):
                            pt = ps_mm.tile([P, 512], F32, tag="mm", name="pmm")[:, : ce - cs]
                            for k in range(KD):
                                wt = wp_tiles[k // wp_half]
                                nc.tensor.matmul(pt[:], xqo[t][:, k, :],
                                                 wt[:, k % wp_half, cs:ce],
                                                 start=(k == 0), stop=(k == KD - 1))
                            # fused evac+residual: z += c * psum
                            nc.vector.scalar_tensor_tensor(
                                z[:, t, cs:ce], pt[:], cps[t][:], z[:, t, cs:ce],
                                OP.mult, OP.add)
                        n2a(t)
                        n2b(t)
                        n2c(t)

                # --- phase 5: fc1 + gelu + g-quant ---
                wf2_tiles, wf2_half = load_w(wfc2_d, b, KH, DIM, 3)
                xqg = [None] * NT
                c4s = [None] * NT
                gs = [None] * NT

                def gquant(t):
                    gh0, gh1 = gs[t]
                    am = scp.tile([P, 1], F32, tag="sc", name="am0")
                    nc.vector.tensor_reduce(am[:], gh0[:], axis=AX.X, op=OP.max,
                                            apply_absolute_value=True)
                    am1 = scp.tile([P, 1], F32, tag="sc", name="am1")
                    nc.vector.tensor_reduce(am1[:], gh1[:], axis=AX.X, op=OP.max,
                                            apply_absolute_value=True)
                    ac = scp.tile([P, 1], F32, tag="sc", name="amaxc")
                    nc.vector.tensor_tensor(ac[:], am[:], am1[:], OP.max)
                    ac2 = scp.tile([P, 1], F32, tag="sc", name="amaxc2")
                    nc.vector.tensor_scalar_max(ac2[:], ac[:], 1e-5)
                    rs = scp.tile([P, 1], F32, tag="sc", name="rcp")
                    nc.vector.reciprocal(rs[:], ac2[:])
                    s127 = scp.tile([P, 1], F32, tag="sc", name="s127")
                    nc.vector.tensor_scalar_mul(s127[:], rs[:], 127.0)
                    c = scp.tile([P, 1], F32, tag="sc", name="cc")
                    nc.vector.tensor_scalar(c[:], ac2[:], wsb[:, 4 * b + 3:4 * b + 4],
                                            None, OP.mult)
                    c4s[t] = c
                    xqg[t] = quant_round_dma_hid(gs[t], s127)

                with nc.named_scope(f"b{b}_fc1"):
                    for t in range(NT):
                        gh0 = gp.tile([P, HID // 2], F32, tag="g")
                        gh1 = gp.tile([P, HID // 2], F32, tag="g")
                        gs[t] = (gh0, gh1)
                        for ci, (cs, ce) in enumerate(_mm_chunks(HID)):
                            pt = ps_mm.tile([P, 512], F32, tag="mm", name="pmm")[:, : ce - cs]
                            for k in range(KD):
                                wt = wf1_tiles[k // wf1_half]
                                nc.tensor.matmul(pt[:], xq2[t][:, k, :],
                                                 wt[:, k % wf1_half, cs:ce],
                                                 start=(k == 0), stop=(k == KD - 1))
                            gh = gh0 if ci < 3 else gh1
                            off = cs - (0 if ci < 3 else HID // 2)
                            nc.scalar.activation(gh[:, off:off + ce - cs], pt[:],
                                                 AF.Gelu_apprx_tanh, scale=c3s[t][:])
                        if t > 0:
                            gquant(t - 1)
                    gquant(NT - 1)

                # --- phase 6: fc2 + residual, fused with next block's phase 1 ---
                fuse = b + 1 < depth
                if fuse:
                    mt1_nxt = load_mods(b + 1, 0, "mt1")
                    xq1_cur = [None] * NT
                    cq8_cur = [None] * NT
                    c_cur = [None] * NT
                    ssq_n = [None] * NT
                    h1s = [None] * NT
                    s127_n = [None] * NT

                def p1a(t):
                    sq = tmp_.tile([P, DIM], F32, tag="tm", name="sqn")
                    sv = scp.tile([P, 1], F32, tag="sc", name="ssqn")
                    nc.scalar.activation(sq[:], z[:, t, :], AF.Square, accum_out=sv[:])
                    ssq_n[t] = sv

                def p1b(t):
                    rst = rstd_from_ssq(ssq_n[t])
                    h = hp.tile([P, DIM], F32, tag="h")
                    h1s[t] = h
                    norm_mod(t, mt1_nxt, rst, h[:])
                    s127, c = quant_smalls(h[:], 4 * (b + 1))
                    c_cur[t] = c
                    cq8 = scp.tile([P, 1], F32, tag="sc", name="cq8")
                    nc.vector.tensor_scalar_mul(cq8[:], c[:], 0.125)
                    cq8_cur[t] = cq8
                    s127_n[t] = s127

                def p1c(t):
                    xq1_cur[t] = quant_round_dma(h1s[t][:], KD, s127_n[t])

                with nc.named_scope(f"b{b}_fc2"):
                    for t in range(NT):
                        for (cs, ce) in _mm_chunks(DIM):
                            pt = ps_mm.tile([P, 512], F32, tag="mm", name="pmm")[:, : ce - cs]
                            for k in range(KH):
                                wt = wf2_tiles[k // wf2_half]
                                nc.tensor.matmul(pt[:], xqg[t][:, k, :],
                                                 wt[:, k % wf2_half, cs:ce],
                                                 start=(k == 0), stop=(k == KH - 1))
                            nc.vector.scalar_tensor_tensor(
                                z[:, t, cs:ce], pt[:], c4s[t][:], z[:, t, cs:ce],
                                OP.mult, OP.add)
                        if fuse:
                            p1a(t)
                            p1b(t)
                            p1c(t)

            # ---------------- final norm + head ----------------
            with nc.named_scope("head"):
                hw_pieces = []
                for i in range(3):
                    hwp = gp.tile([P, 2, DIM], F32, tag="g", name="hwp")
                    nc.gpsimd.dma_start(
                        hwp[:], headWT_d[i * 2 * P:(i + 1) * 2 * P, :].rearrange(
                            "(o p) d -> p o d", p=P))
                    hw_pieces.append(hwp)
                hbrow = tmp_.tile([1, DIM], F32, tag="tm", name="hbrow")
                nc.sync.dma_start(hbrow[:], headb_d[:])
                hbb = gp.tile([P, DIM], F32, tag="g", name="hbb")
                nc.gpsimd.partition_broadcast(hbb[:], hbrow[0:1, :])
                ssq_f = [None] * NT
                sqf = tmp_.tile([P, DIM], F32, tag="tm", name="sqf")
                for t in range(NT):
                    sv = scp.tile([P, 1], F32, tag="sc", name="ssqf")
                    nc.scalar.activation(sqf[:], z[:, t, :], AF.Square, accum_out=sv[:])
                    ssq_f[t] = sv
                for t in range(NT):
                    rst = rstd_from_ssq(ssq_f[t])
                    zn = hp.tile([P, DIM], F32, tag="h")
                    nc.vector.tensor_scalar_mul(zn[:], z[:, t, :], rst[:])
                    znT = hp.tile([P, DIM], F32, tag="h")
                    for g0 in range(0, KD, 4):
                        gn = min(4, KD - g0)
                        ptb = ps_tp.tile([P, 512], F32, tag="tp", name="ptb")[:, : gn * P]
                        for j in range(gn):
                            nc.tensor.transpose(ptb[:, j * P:(j + 1) * P],
                                                zn[:, (g0 + j) * P:(g0 + j + 1) * P], idf[:])
                        nc.vector.tensor_copy(znT[:, g0 * P:(g0 + gn) * P], ptb[:])
                    for (cs, ce) in _mm_chunks(DIM):
                        pt = ps_mm.tile([P, 512], F32, tag="mm", name="pmm")[:, : ce - cs]
                        for k in range(KD):
                            nc.tensor.matmul(pt[:], znT[:, k * P:(k + 1) * P],
                                             hw_pieces[k // 2][:, k % 2, cs:ce],
                                             start=(k == 0), stop=(k == KD - 1))
                        ot = tmp_.tile([P, DIM], F32, tag="tm", name="ot")[:, : ce - cs]
                        nc.vector.tensor_tensor(ot[:], pt[:], hbb[:, cs:ce], OP.add)
                        nc.sync.dma_start(out_d[t * P:(t + 1) * P, cs:ce], ot[:])

    nc.compile()
    return nc


# ---------------------------------------------------------------------------
# host-side numerics (numpy, fp32 — matches jax CPU within ~1e-7)

def _gelu_tanh(x):
    x = x.astype(np.float32)
    c = np.float32(math.sqrt(2.0 / math.pi))
    return np.float32(0.5) * x * (np.float32(1.0) +
                                  np.tanh(c * (x + np.float32(0.044715) * x * x * x)))


def _time_embedding(t, t_w1, t_b1, t_w2, t_b2):
    half = DIM // 2
    freqs = np.exp(-np.log(10000.0) * np.arange(half, dtype=np.float32) / (half - 1)).astype(np.float32)
    args = t[:, None].astype(np.float32) * freqs[None, :]
    emb = np.concatenate([np.sin(args), np.cos(args)], axis=-1).astype(np.float32)
    h = _gelu_tanh(emb @ t_w1.T + t_b1)
    return (h @ t_w2.T + t_b2).astype(np.float32)


def _quant_w(w):
    ws = np.float32(np.mean(np.abs(w), dtype=np.float64)) + np.float32(1e-5)
    wq = np.clip(np.round(w.astype(np.float32) / ws), -1.0, 1.0)
    return wq, ws


def _prepare(inputs):
    x = np.asarray(inputs["x"], np.float32)
    t = np.asarray(inputs["t"], np.float32)
    B = x.shape[0]
    n_cores = 8
    per = B // n_cores  # 2
    p = PATCH
    hh = IMG // p

    xp = x.reshape(B, CIN, hh, p, hh, p).transpose(0, 2, 4, 1, 3, 5).reshape(B, hh * hh, CIN * p * p)

    t_emb = _time_embedding(t, inputs["t_w1"], inputs["t_b1"], inputs["t_w2"], inputs["t_b2"])
    silu = (t_emb / (1.0 + np.exp(-t_emb))).astype(np.float32)

    depth = DEPTH
    mods = np.zeros((depth, 2, B, 2, DIM), np.float32)  # [blk, norm, img, A/B, D]
    wscl = np.zeros((4 * depth,), np.float32)
    wq_all, wp_all, wf1_all, wf2_all = [], [], [], []
    for b in range(depth):
        mod = silu @ np.asarray(inputs["blk_ada_w"][b], np.float32).T + np.asarray(
            inputs["blk_ada_b"][b], np.float32)
        sh1, sc1, sh2, sc2 = np.split(mod, 4, axis=-1)
        n1 = np.asarray(inputs["blk_norm1"][b], np.float32)
        n2 = np.asarray(inputs["blk_norm2"][b], np.float32)
        mods[b, 0, :, 0, :] = n1[None, :] * (1.0 + sc1)
        mods[b, 0, :, 1, :] = sh1
        mods[b, 1, :, 0, :] = n2[None, :] * (1.0 + sc2)
        mods[b, 1, :, 1, :] = sh2

        for j, (nm, lst) in enumerate([("blk_qkv", wq_all), ("blk_proj", wp_all),
                                       ("blk_fc1", wf1_all), ("blk_fc2", wf2_all)]):
            wq, ws = _quant_w(np.asarray(inputs[nm][b], np.float32))
            lst.append(np.ascontiguousarray(wq.T).astype(ml_dtypes.float8_e4m3))
            wscl[4 * b + j] = ws / np.float32(127.0)

    wqkv = np.stack(wq_all)
    wproj = np.stack(wp_all)
    wfc1 = np.stack(wf1_all)
    wfc2 = np.stack(wf2_all)

    posb = (np.asarray(inputs["pos_embed"][0], np.float32) +
            np.asarray(inputs["patch_b"], np.float32)[None, :]).astype(np.float32)
    patchWT = np.ascontiguousarray(np.asarray(inputs["patch_w"], np.float32).T)
    norm_w = np.asarray(inputs["norm_w"], np.float32)
    headWT = np.ascontiguousarray(np.asarray(inputs["head_w"], np.float32).T * norm_w[:, None])
    headb = np.asarray(inputs["head_b"], np.float32)[None, :]

    key = ("prog", depth)
    if key not in _CACHED:
        _CACHED[key] = build_program(depth)
    nc = _CACHED[key]

    in_maps = []
    for c in range(n_cores):
        imgs = slice(c * per, (c + 1) * per)
        xpT = np.ascontiguousarray(xp[imgs].reshape(per * hh * hh, CIN * p * p).T)
        in_maps.append(dict(
            xpT=xpT, posb=posb, patchWT=patchWT, headWT=headWT, headb=headb,
            wqkv=wqkv, wproj=wproj, wfc1=wfc1, wfc2=wfc2,
            mods=np.ascontiguousarray(
                np.broadcast_to(mods[:, :, None, imgs], (depth, 2, 128, per, 2, DIM))),
            wscl=wscl[None, :],
        ))

    return nc, in_maps


def _assemble(res, B=16, per=2):
    p = PATCH
    hh = IMG // p
    out = np.zeros((B, CIN, IMG, IMG), np.float32)
    for c in range(B // per):
        zo = res.results[c]["zout"]  # [512, 768]
        for i in range(per):
            zi = zo[i * 256:(i + 1) * 256]
            out[c * per + i] = zi.reshape(hh, hh, CIN, p, p).transpose(2, 0, 3, 1, 4).reshape(CIN, IMG, IMG)
    return out


def kernel(**inputs):
    nc, in_maps = _prepare(inputs)
    res = run_bass_kernel_spmd(nc, in_maps, list(range(len(in_maps))), trace=False)
    return _assemble(res)



# revision 22
# speedup vs baseline: 1.0339x; 1.0339x over previous
"""BitNet DiT on 8 Trainium2 NeuronCores — data-parallel over batch (2 images/core).

Host: patchify, time-embedding + adaLN modulation vectors, BitNet weight
quantization (ternary * per-tensor scale) -> fp8 upload.
Device: full 12-block DiT forward per core in a single Bass/Tile kernel.
BitNet matmuls run as exact integer arithmetic in bf16 (|values| <= 127,
fp32 accumulate). Attention runs via transposed-logits + ones-column
softmax-denominator trick.

v3 (vs v2):
- quant chains (ssq, rstd, amax, magic rounds) moved to DVE; rstd via
  Newton-Raphson rsqrt (bit trick + 2 iters) -> no Sqrt ACT table loads
  (only 2 table switches/block: exp <-> gelu).
- q/k computed weight-stationary producing [feat, tok] layout directly:
  kills 48 PE transposes + ACT evacs per block. Per-token activation
  scales applied via a broadcast c-row tile (PE column-transpose trick).
- attention softmax normalization batched: 6 heads per PSUM tile, one
  strided reciprocal + one broadcast multiply (was 48 ACT ops/block).
- softmax weights (eT) in bf16: halves SBUF + faster AV LDWEIGHTS.
- phase order tuned so PE never idles >3.4us (HAM stays at 2.4 GHz):
  v-matmuls before q/k, o-quant interleaved with attention tail.
"""
import math
import os
import sys
import numpy as np

sys.path.insert(0, "/opt/trn_rl_repo")

import ml_dtypes  # noqa: E402
import concourse.bass as bass  # noqa: E402
import concourse.mybir as mybir  # noqa: E402
import concourse.tile as tile  # noqa: E402
from concourse import bacc  # noqa: E402
from concourse.bass_utils import run_bass_kernel_spmd  # noqa: E402
from concourse.masks import make_identity  # noqa: E402

F32 = mybir.dt.float32
F32R = mybir.dt.float32r
I32 = mybir.dt.int32
FP8 = mybir.dt.float8e4
BF16 = mybir.dt.bfloat16
AX = mybir.AxisListType
OP = mybir.AluOpType
AF = mybir.ActivationFunctionType

DIM = 768
DEPTH = int(os.environ.get("KERNEL_DEPTH", "12"))
HEADS = 12
HD = 64
PATCH = 16
IMG = 256
CIN = 3
HID = 4 * DIM
EPS = 1e-6
P = 128
T = 512            # tokens per core (2 images x 256)
NT = T // P        # 4 token tiles
NTOK = 256         # tokens per image
KD = DIM // P      # 6
KH = HID // P      # 24
MAGIC = float(np.float32(3 * 2**22))  # 12582912.0 RNE round-to-int magic
RSQRT_SEED = 0x5F375A86

_CACHED = {}


def _mm_chunks(n):
    out = []
    s = 0
    while s < n:
        e = min(s + 512, n)
        out.append((s, e))
        s = e
    return out


def build_program(depth=DEPTH):
    nc = bacc.Bacc("TRN2", target_bir_lowering=False, debug=False, num_devices=8)

    xpT_d = nc.declare_dram_parameter("xpT", [DIM, T], F32, isOutput=False)
    posb_d = nc.declare_dram_parameter("posb", [NTOK, DIM], F32, isOutput=False)
    patchWT_d = nc.declare_dram_parameter("patchWT", [DIM, DIM], F32, isOutput=False)
    headWT_d = nc.declare_dram_parameter("headWT", [DIM, DIM], F32, isOutput=False)
    headb_d = nc.declare_dram_parameter("headb", [1, DIM], F32, isOutput=False)
    wqkv_d = nc.declare_dram_parameter("wqkv", [depth, DIM, 3 * DIM], FP8, isOutput=False)
    wproj_d = nc.declare_dram_parameter("wproj", [depth, DIM, DIM], FP8, isOutput=False)
    wfc1_d = nc.declare_dram_parameter("wfc1", [depth, DIM, HID], FP8, isOutput=False)
    wfc2_d = nc.declare_dram_parameter("wfc2", [depth, HID, DIM], FP8, isOutput=False)
    # modulation vectors: [block, norm(2), part, img(2), A/B(2), 768]
    mods_d = nc.declare_dram_parameter("mods", [depth, 2, P, 2, 2, DIM], F32, isOutput=False)
    wscl_d = nc.declare_dram_parameter("wscl", [1, 4 * depth], F32, isOutput=False)
    out_d = nc.declare_dram_parameter("zout", [T, DIM], F32, isOutput=True)

    with tile.TileContext(nc) as tc:
        from contextlib import ExitStack
        with ExitStack() as _ctx:
            constp = _ctx.enter_context(tc.tile_pool(name="const", bufs=1))
            residp = _ctx.enter_context(tc.tile_pool(name="resid", bufs=1))
            qkp = _ctx.enter_context(tc.tile_pool(name="qk", bufs=2))
            wp = _ctx.enter_context(tc.tile_pool(name="w", bufs=6))
            modp = _ctx.enter_context(tc.tile_pool(name="mod", bufs=2))
            tmp_ = _ctx.enter_context(tc.tile_pool(name="tm", bufs=2))
            gp = _ctx.enter_context(tc.tile_pool(name="g", bufs=3))
            hp = _ctx.enter_context(tc.tile_pool(name="h", bufs=2))
            xqtp = _ctx.enter_context(tc.tile_pool(name="xqt", bufs=2))
            xqop = _ctx.enter_context(tc.tile_pool(name="xqo", bufs=1))
            xq2p = _ctx.enter_context(tc.tile_pool(name="xq2", bufs=1))
            xqgp = _ctx.enter_context(tc.tile_pool(name="xqg", bufs=2))
            xqsp = _ctx.enter_context(tc.tile_pool(name="xqs", bufs=2))
            eTp = _ctx.enter_context(tc.tile_pool(name="eT", bufs=1))
            cbp = _ctx.enter_context(tc.tile_pool(name="cb", bufs=2))
            scp = _ctx.enter_context(tc.tile_pool(name="sc", bufs=64))
            ps_mm = _ctx.enter_context(tc.tile_pool(name="ps_mm", bufs=2, space="PSUM"))
            ps_lt = _ctx.enter_context(tc.tile_pool(name="ps_lt", bufs=2, space="PSUM"))
            ps_oa = _ctx.enter_context(tc.tile_pool(name="ps_oa", bufs=2, space="PSUM"))
            ps_x = _ctx.enter_context(tc.tile_pool(name="ps_x", bufs=1, space="PSUM"))

            idf = constp.tile([P, P], F32)
            make_identity(nc, idf[:])

            # broadcast w_scales/127 to all partitions
            wsrow = constp.tile([1, 4 * depth], F32)
            nc.sync.dma_start(wsrow[:], wscl_d[:])
            wsb = constp.tile([P, 4 * depth], F32)
            nc.gpsimd.partition_broadcast(wsb[:], wsrow[0:1, :])
            pmag = constp.tile([P, 1], F32)
            nc.vector.memset(pmag[:], MAGIC)

            z = residp.tile([P, NT, DIM], F32)
            v_aug = residp.tile([P, NT, HEADS, HD + 1], BF16)
            nc.vector.memset(v_aug[:, :, :, HD], 1.0)
            o_tm = residp.tile([P, NT, DIM], F32)

            # ---------- tiny DVE helpers (no ACT involvement) ----------
            def rstd_dve(ssq):
                """rstd = 1/sqrt(ssq/DIM + EPS) via bit-trick + 2 Newton iters."""
                ms = scp.tile([P, 1], F32, tag="sc", name="ms")
                nc.vector.tensor_scalar(ms[:], ssq[:], 1.0 / DIM, EPS, OP.mult, OP.add)
                hx = scp.tile([P, 1], F32, tag="sc", name="hx")
                nc.vector.tensor_scalar_mul(hx[:], ms[:], 0.5)
                yi = scp.tile([P, 1], I32, tag="sc", name="yi")
                nc.vector.tensor_scalar(yi[:], ms[:].bitcast(I32), 1, None,
                                        OP.arith_shift_right)
                yn = scp.tile([P, 1], I32, tag="sc", name="yn")
                nc.vector.tensor_scalar(yn[:], yi[:], -1, None, OP.bitwise_xor)
                y = scp.tile([P, 1], I32, tag="sc", name="y0")
                nc.vector.tensor_scalar(y[:], yn[:], RSQRT_SEED + 1, None, OP.add)
                yf = y[:].bitcast(F32)
                for it in range(2):
                    a = scp.tile([P, 1], F32, tag="sc", name=f"nra{it}")
                    nc.vector.tensor_tensor(a[:], yf, hx[:], OP.mult)
                    b = scp.tile([P, 1], F32, tag="sc", name=f"nrb{it}")
                    nc.vector.tensor_tensor(b[:], a[:], yf, OP.mult)
                    c = scp.tile([P, 1], F32, tag="sc", name=f"nrc{it}")
                    nc.vector.tensor_scalar(c[:], b[:], -1.0, 1.5, OP.mult, OP.add)
                    y2 = scp.tile([P, 1], F32, tag="sc", name=f"nry{it}")
                    nc.vector.tensor_tensor(y2[:], yf, c[:], OP.mult)
                    yf = y2[:]
                return yf

            sq_scr = tmp_.tile([P, DIM], F32, tag="tm", name="sqscr")

            def ssq_dve(src_ap):
                sv = scp.tile([P, 1], F32, tag="sc", name="ssq")
                nc.vector.scalar_tensor_tensor(sq_scr[:], src_ap, 1.0, src_ap,
                                               OP.bypass, OP.mult, accum_out=sv[:])
                return sv

            def norm_mod(t, mt, rstd_ap, dst):
                """dst = (z[t]*rstd) * modA + modB  (2 DVE ops)."""
                img = t // 2
                nc.vector.scalar_tensor_tensor(dst, z[:, t, :], rstd_ap,
                                               mt[:, img, 0, :], OP.mult, OP.mult)
                nc.vector.tensor_tensor(dst, dst, mt[:, img, 1, :], OP.add)

            def quant_scales(src_ap, ws_idx):
                """amax over src rows -> (s127, c). All DVE, no 1e-5 clamp
                (amax >> 1e-5 always for this net)."""
                amax = scp.tile([P, 1], F32, tag="sc", name="amax")
                nc.vector.tensor_reduce(amax[:], src_ap, axis=AX.X, op=OP.max,
                                        apply_absolute_value=True)
                rs = scp.tile([P, 1], F32, tag="sc", name="rcp")
                nc.vector.reciprocal(rs[:], amax[:])
                s127 = scp.tile([P, 1], F32, tag="sc", name="s127")
                nc.vector.tensor_scalar_mul(s127[:], rs[:], 127.0)
                c = scp.tile([P, 1], F32, tag="sc", name="cc")
                nc.vector.tensor_scalar(c[:], amax[:], wsb[:, ws_idx:ws_idx + 1],
                                        None, OP.mult)
                return s127, c

            def round_dma(src_ap, s127, dst_slice, width):
                """in-place magic-round src*s127, unmagic to bf16 ints,
                DMA-transpose into dst."""
                nc.vector.tensor_scalar(src_ap, src_ap, s127[:], MAGIC, OP.mult, OP.add)
                xq = xqsp.tile([P, width], BF16, tag="xqs", name="xqs")
                nc.vector.tensor_scalar(xq[:], src_ap, MAGIC, None, OP.subtract)
                nc.sync.dma_start_transpose(dst_slice, xq[:])

            # ---------------- patch embed ----------------
            posb_sb = wp.tile([P, 2, DIM], F32, tag="w", name="posb_sb")
            nc.sync.dma_start(posb_sb[:], posb_d.rearrange("(a p) d -> p a d", p=P))
            xpT = qkp.tile([P, KD, T], F32, tag="qk")
            nc.sync.dma_start(xpT[:], xpT_d.rearrange("(o p) t -> p o t", p=P))
            pw_pieces = []
            for i in range(3):
                pwp = wp.tile([P, 2, DIM], F32, tag="w", name="pwp")
                nc.gpsimd.dma_start(
                    pwp[:], patchWT_d[i * 2 * P:(i + 1) * 2 * P, :].rearrange(
                        "(o p) d -> p o d", p=P))
                pw_pieces.append(pwp)
            for t in range(NT):
                for (cs, ce) in _mm_chunks(DIM):
                    pt = ps_mm.tile([P, 512], F32, tag="mm", name="pmm")[:, : ce - cs]
                    for k in range(KD):
                        nc.tensor.matmul(pt[:], xpT[:, k, t * P:(t + 1) * P],
                                         pw_pieces[k // 2][:, k % 2, cs:ce],
                                         start=(k == 0), stop=(k == KD - 1))
                    nc.vector.tensor_tensor(z[:, t, cs:ce], pt[:], posb_sb[:, t % 2, cs:ce], OP.add)

            def load_w(dram, b, kchunks, width, npieces):
                """Stage one linear's transposed fp8 weights as npieces tiles."""
                span = kchunks // npieces
                tiles = []
                for i in range(npieces):
                    wt = wp.tile([P, span, width], FP8, tag="w")
                    nc.gpsimd.dma_start(
                        wt[:],
                        dram[b, i * span * P:(i + 1) * span * P, :].rearrange(
                            "(o p) f -> p o f", p=P))
                    tiles.append(wt)
                return tiles, span

            def load_mods(b_, n_, name):
                mt = modp.tile([P, 2, 2, DIM], F32, tag="mod", name=name)
                nc.gpsimd.dma_start(mt[:], mods_d[b_, n_])
                return mt

            # ---- prologue: phase 1 of block 0 ----
            mt1_nxt = load_mods(0, 0, "mt1")
            xqT_cur = xqtp.tile([P, KD, T], BF16, tag="xqt")
            c_cur = [None] * NT
            for t in range(NT):
                rst = rstd_dve(ssq_dve(z[:, t, :]))
                h = hp.tile([P, DIM], F32, tag="h")
                norm_mod(t, mt1_nxt, rst, h[:])
                s127, c = quant_scales(h[:], 0)
                c_cur[t] = c
                round_dma(h[:], s127, xqT_cur[:, :, t * P:(t + 1) * P], DIM)

            for b in range(depth):
                xqT, c_list = xqT_cur, c_cur
                mt2 = load_mods(b, 1, "mt2")
                wq_tiles, wq_span = load_w(wqkv_d, b, KD, 3 * DIM, 3)

                with nc.named_scope(f"b{b}_qkv"):
                    # c-row: transpose per-token scales into a [1,T] row, then
                    # broadcast to all partitions for the q/k evac scaling.
                    crow_ps = ps_x.tile([1, NT, P], F32, tag="x", name="crow_ps")
                    for t in range(NT):
                        nc.tensor.transpose(crow_ps[:, t, :], c_list[t][:], idf[:])
                    crow = cbp.tile([1, T], F32, tag="cb", name="crow")
                    nc.vector.tensor_copy(crow[:], crow_ps[:].rearrange("a b c -> a (b c)"))
                    cb = cbp.tile([P, T], F32, tag="cb", name="cbb")
                    nc.gpsimd.partition_broadcast(cb[:], crow[0:1, :])

                    # v first (per-tile deps) keeps PE fed while last p1 lands
                    for t in range(NT):
                        for ci, (cs, ce) in enumerate(_mm_chunks(DIM)):
                            pt = ps_mm.tile([P, 512], F32, tag="mm", name="pmm")[:, : ce - cs]
                            for k in range(KD):
                                wt = wq_tiles[k // wq_span]
                                nc.tensor.matmul(
                                    pt[:], xqT[:, k, t * P:(t + 1) * P],
                                    wt[:, k % wq_span, 2 * DIM + cs:2 * DIM + ce],
                                    start=(k == 0), stop=(k == KD - 1))
                            h0 = cs // HD
                            h1 = ce // HD
                            nc.scalar.activation(v_aug[:, t, h0:h1, 0:HD], pt[:],
                                                 AF.Identity, scale=c_list[t][:])
                    # q/k weight-stationary: psum [feat 128, tok 512]
                    for fc in range(12):
                        pt = ps_mm.tile([P, 512], F32, tag="mm", name="pmm")
                        for k in range(KD):
                            wt = wq_tiles[k // wq_span]
                            nc.tensor.matmul(
                                pt[:], wt[:, k % wq_span, fc * P:(fc + 1) * P],
                                xqT[:, k, :], start=(k == 0), stop=(k == KD - 1))
                        if fc < 6:
                            q_fm = qkp.tile([P, KD, T], BF16, tag="qk", name="q_fm") \
                                if fc == 0 else q_fm
                            nc.vector.scalar_tensor_tensor(
                                q_fm[:, fc, :], pt[:], 0.125, cb[:], OP.mult, OP.mult)
                        else:
                            k_fm = qkp.tile([P, KD, T], BF16, tag="qk", name="k_fm") \
                                if fc == 6 else k_fm
                            nc.vector.tensor_tensor(k_fm[:, fc - 6, :], pt[:], cb[:],
                                                    OP.mult)

                # --- attention ---
                wp_tiles, wp_span = load_w(wproj_d, b, KD, DIM, 2)
                xqoT = xqop.tile([P, KD, T], BF16, tag="xqo")
                co_s = [None] * NT

                def o_quant(t):
                    s127, c = quant_scales(o_tm[:, t, :], 4 * b + 1)
                    co_s[t] = c
                    round_dma(o_tm[:, t, :], s127, xqoT[:, :, t * P:(t + 1) * P], DIM)

                with nc.named_scope(f"b{b}_attn"):
                    for img in range(2):
                        for g in range(2):
                            eTg = eTp.tile([P, 6, 2, NTOK], BF16, tag="eT")
                            for h6 in range(6):
                                hh = 6 * g + h6
                                po = (hh % 2) * HD
                                ch = hh // 2
                                lt = ps_lt.tile([P, 2, NTOK], F32, tag="lt")
                                for mt in range(2):
                                    nc.tensor.matmul(
                                        lt[:, mt, :],
                                        k_fm[po:po + HD, ch,
                                             img * NTOK + mt * P: img * NTOK + (mt + 1) * P],
                                        q_fm[po:po + HD, ch,
                                             img * NTOK: (img + 1) * NTOK],
                                        start=True, stop=True)
                                nc.scalar.activation(eTg[:, h6], lt[:], AF.Exp)
                            for nt in range(2):
                                oa = ps_oa.tile([P, 6, HD + 1], F32, tag="oa")
                                for h6 in range(6):
                                    for mt in range(2):
                                        nc.tensor.matmul(
                                            oa[:, h6, :],
                                            eTg[:, h6, mt, nt * P:(nt + 1) * P],
                                            v_aug[:, img * 2 + mt, 6 * g + h6, :],
                                            start=(mt == 0), stop=(mt == 1))
                                rinv = scp.tile([P, 6], F32, tag="sc", name="rinv")
                                nc.vector.reciprocal(rinv[:], oa[:, :, HD])
                                dst = o_tm[:, img * 2 + nt,
                                           384 * g:384 * (g + 1)].rearrange(
                                               "p (h d) -> p h d", h=6)
                                nc.vector.tensor_tensor(
                                    dst, oa[:, :, 0:HD],
                                    rinv[:, :, None].broadcast_to([P, 6, HD]),
                                    OP.mult)
                            if g == 1:
                                o_quant(img * 2 + 0)
                                o_quant(img * 2 + 1)

                # --- proj + n2 chain ---
                wf1_tiles, wf1_span = load_w(wfc1_d, b, KD, HID, 3)
                xq2T = xq2p.tile([P, KD, T], BF16, tag="xq2")
                c3s = [None] * NT

                def n2(t):
                    rst = rstd_dve(ssq_dve(z[:, t, :]))
                    h = hp.tile([P, DIM], F32, tag="h")
                    norm_mod(t, mt2, rst, h[:])
                    s127, c = quant_scales(h[:], 4 * b + 2)
                    c3s[t] = c
                    round_dma(h[:], s127, xq2T[:, :, t * P:(t + 1) * P], DIM)

                with nc.named_scope(f"b{b}_proj"):
                    for t in range(NT):
                        for (cs, ce) in _mm_chunks(DIM):
                            pt = ps_mm.tile([P, 512], F32, tag="mm", name="pmm")[:, : ce - cs]
                            for k in range(KD):
                                wt = wp_tiles[k // wp_span]
                                nc.tensor.matmul(pt[:], xqoT[:, k, t * P:(t + 1) * P],
                                                 wt[:, k % wp_span, cs:ce],
                                                 start=(k == 0), stop=(k == KD - 1))
                            nc.vector.scalar_tensor_tensor(
                                z[:, t, cs:ce], pt[:], co_s[t][:], z[:, t, cs:ce],
                                OP.mult, OP.add)
                        n2(t)

                # --- fc1 + gelu + g-quant ---
                wf2_tiles, wf2_span = load_w(wfc2_d, b, KH, DIM, 3)
                xqg = [None] * NT
                c4s = [None] * NT
                gs = [None] * NT

                def gquant(t):
                    gh0, gh1 = gs[t]
                    am = scp.tile([P, 1], F32, tag="sc", name="am0")
                    nc.vector.tensor_reduce(am[:], gh0[:], axis=AX.X, op=OP.max,
                                            apply_absolute_value=True)
                    am1 = scp.tile([P, 1], F32, tag="sc", name="am1")
                    nc.vector.tensor_reduce(am1[:], gh1[:], axis=AX.X, op=OP.max,
                                            apply_absolute_value=True)
                    ac = scp.tile([P, 1], F32, tag="sc", name="amaxc")
                    nc.vector.tensor_tensor(ac[:], am[:], am1[:], OP.max)
                    rs = scp.tile([P, 1], F32, tag="sc", name="rcp")
                    nc.vector.reciprocal(rs[:], ac[:])
                    s127 = scp.tile([P, 1], F32, tag="sc", name="s127")
                    nc.vector.tensor_scalar_mul(s127[:], rs[:], 127.0)
                    c = scp.tile([P, 1], F32, tag="sc", name="cc")
                    nc.vector.tensor_scalar(c[:], ac[:], wsb[:, 4 * b + 3:4 * b + 4],
                                            None, OP.mult)
                    c4s[t] = c
                    xg = xqgp.tile([P, KH, P], BF16, tag="xqg")
                    xqg[t] = xg
                    for i, gh in enumerate((gh0, gh1)):
                        # magic on ACT (Identity, no table switch), unmagic DVE
                        nc.scalar.activation(gh[:], gh[:], AF.Identity,
                                             scale=s127[:], bias=pmag[:])
                        xq = xqsp.tile([P, HID // 2], BF16, tag="xqs", name="xq24s")
                        nc.vector.tensor_scalar(xq[:], gh[:], MAGIC, None, OP.subtract)
                        nc.sync.dma_start_transpose(xg[:, i * 12:(i + 1) * 12, :], xq[:])

                # --- fc1/fc2 interleaved per tile, fc2 fused with next p1 ---
                fuse = b + 1 < depth
                if fuse:
                    mt1_nxt = load_mods(b + 1, 0, "mt1")
                    xqT_cur = xqtp.tile([P, KD, T], BF16, tag="xqt")
                    c_cur = [None] * NT

                def p1(t):
                    rst = rstd_dve(ssq_dve(z[:, t, :]))
                    h = hp.tile([P, DIM], F32, tag="h")
                    norm_mod(t, mt1_nxt, rst, h[:])
                    s127, c = quant_scales(h[:], 4 * (b + 1))
                    c_cur[t] = c
                    round_dma(h[:], s127, xqT_cur[:, :, t * P:(t + 1) * P], DIM)

                def fc1_t(t):
                    gh0 = gp.tile([P, HID // 2], F32, tag="g")
                    gh1 = gp.tile([P, HID // 2], F32, tag="g")
                    gs[t] = (gh0, gh1)
                    for ci, (cs, ce) in enumerate(_mm_chunks(HID)):
                        pt = ps_mm.tile([P, 512], F32, tag="mm", name="pmm")[:, : ce - cs]
                        for k in range(KD):
                            wt = wf1_tiles[k // wf1_span]
                            nc.tensor.matmul(pt[:], xq2T[:, k, t * P:(t + 1) * P],
                                             wt[:, k % wf1_span, cs:ce],
                                             start=(k == 0), stop=(k == KD - 1))
                        gh = gh0 if ci < 3 else gh1
                        off = cs - (0 if ci < 3 else HID // 2)
                        nc.scalar.activation(gh[:, off:off + ce - cs], pt[:],
                                             AF.Gelu_apprx_tanh, scale=c3s[t][:])

                def fc2_t(t):
                    for (cs, ce) in _mm_chunks(DIM):
                        pt = ps_mm.tile([P, 512], F32, tag="mm", name="pmm")[:, : ce - cs]
                        for k in range(KH):
                            wt = wf2_tiles[k // wf2_span]
                            nc.tensor.matmul(pt[:], xqg[t][:, k, :],
                                             wt[:, k % wf2_span, cs:ce],
                                             start=(k == 0), stop=(k == KH - 1))
                        nc.vector.scalar_tensor_tensor(
                            z[:, t, cs:ce], pt[:], c4s[t][:], z[:, t, cs:ce],
                            OP.mult, OP.add)
                    if fuse:
                        p1(t)

                with nc.named_scope(f"b{b}_mlp"):
                    for t in range(NT):
                        fc1_t(t)
                        if t > 0:
                            gquant(t - 1)
                            fc2_t(t - 1)
                    gquant(NT - 1)
                    fc2_t(NT - 1)

            # ---------------- final norm + head ----------------
            with nc.named_scope("head"):
                hw_pieces = []
                for i in range(3):
                    hwp = wp.tile([P, 2, DIM], F32, tag="w", name="hwp")
                    nc.gpsimd.dma_start(
                        hwp[:], headWT_d[i * 2 * P:(i + 1) * 2 * P, :].rearrange(
                            "(o p) d -> p o d", p=P))
                    hw_pieces.append(hwp)
                hbrow = tmp_.tile([1, DIM], F32, tag="tm", name="hbrow")
                nc.sync.dma_start(hbrow[:], headb_d[:])
                hbb = wp.tile([P, DIM], F32, tag="w", name="hbb")
                nc.gpsimd.partition_broadcast(hbb[:], hbrow[0:1, :])
                for t in range(NT):
                    rst = rstd_dve(ssq_dve(z[:, t, :]))
                    zn = hp.tile([P, DIM], F32, tag="h")
                    nc.vector.tensor_scalar_mul(zn[:], z[:, t, :], rst)
                    znT = hp.tile([P, DIM], F32, tag="h")
                    for g0 in range(0, KD, 4):
                        gn = min(4, KD - g0)
                        ptb = ps_lt.tile([P, 512], F32, tag="lt", name="ptb")[:, : gn * P]
                        for j in range(gn):
                            nc.tensor.transpose(ptb[:, j * P:(j + 1) * P],
                                                zn[:, (g0 + j) * P:(g0 + j + 1) * P], idf[:])
                        nc.vector.tensor_copy(znT[:, g0 * P:(g0 + gn) * P], ptb[:])
                    for (cs, ce) in _mm_chunks(DIM):
                        pt = ps_mm.tile([P, 512], F32, tag="mm", name="pmm")[:, : ce - cs]
                        for k in range(KD):
                            nc.tensor.matmul(pt[:], znT[:, k * P:(k + 1) * P],
                                             hw_pieces[k // 2][:, k % 2, cs:ce],
                                             start=(k == 0), stop=(k == KD - 1))
                        ot = tmp_.tile([P, DIM], F32, tag="tm", name="ot")[:, : ce - cs]
                        nc.vector.tensor_tensor(ot[:], pt[:], hbb[:, cs:ce], OP.add)
                        nc.sync.dma_start(out_d[t * P:(t + 1) * P, cs:ce], ot[:])

    nc.compile()
    return nc


# ---------------------------------------------------------------------------
# host-side numerics (numpy, fp32 — matches jax CPU within ~1e-7)

def _gelu_tanh(x):
    x = x.astype(np.float32)
    c = np.float32(math.sqrt(2.0 / math.pi))
    return np.float32(0.5) * x * (np.float32(1.0) +
                                  np.tanh(c * (x + np.float32(0.044715) * x * x * x)))


def _time_embedding(t, t_w1, t_b1, t_w2, t_b2):
    half = DIM // 2
    freqs = np.exp(-np.log(10000.0) * np.arange(half, dtype=np.float32) / (half - 1)).astype(np.float32)
    args = t[:, None].astype(np.float32) * freqs[None, :]
    emb = np.concatenate([np.sin(args), np.cos(args)], axis=-1).astype(np.float32)
    h = _gelu_tanh(emb @ t_w1.T + t_b1)
    return (h @ t_w2.T + t_b2).astype(np.float32)


def _quant_w(w):
    ws = np.float32(np.mean(np.abs(w), dtype=np.float64)) + np.float32(1e-5)
    wq = np.clip(np.round(w.astype(np.float32) / ws), -1.0, 1.0)
    return wq, ws


def _prepare(inputs):
    x = np.asarray(inputs["x"], np.float32)
    t = np.asarray(inputs["t"], np.float32)
    B = x.shape[0]
    n_cores = 8
    per = B // n_cores  # 2
    p = PATCH
    hh = IMG // p

    xp = x.reshape(B, CIN, hh, p, hh, p).transpose(0, 2, 4, 1, 3, 5).reshape(B, hh * hh, CIN * p * p)

    t_emb = _time_embedding(t, inputs["t_w1"], inputs["t_b1"], inputs["t_w2"], inputs["t_b2"])
    silu = (t_emb / (1.0 + np.exp(-t_emb))).astype(np.float32)

    depth = DEPTH
    mods = np.zeros((depth, 2, B, 2, DIM), np.float32)  # [blk, norm, img, A/B, D]
    wscl = np.zeros((4 * depth,), np.float32)
    wq_all, wp_all, wf1_all, wf2_all = [], [], [], []
    for b in range(depth):
        mod = silu @ np.asarray(inputs["blk_ada_w"][b], np.float32).T + np.asarray(
            inputs["blk_ada_b"][b], np.float32)
        sh1, sc1, sh2, sc2 = np.split(mod, 4, axis=-1)
        n1 = np.asarray(inputs["blk_norm1"][b], np.float32)
        n2 = np.asarray(inputs["blk_norm2"][b], np.float32)
        mods[b, 0, :, 0, :] = n1[None, :] * (1.0 + sc1)
        mods[b, 0, :, 1, :] = sh1
        mods[b, 1, :, 0, :] = n2[None, :] * (1.0 + sc2)
        mods[b, 1, :, 1, :] = sh2

        for j, (nm, lst) in enumerate([("blk_qkv", wq_all), ("blk_proj", wp_all),
                                       ("blk_fc1", wf1_all), ("blk_fc2", wf2_all)]):
            wq, ws = _quant_w(np.asarray(inputs[nm][b], np.float32))
            lst.append(np.ascontiguousarray(wq.T).astype(ml_dtypes.float8_e4m3))
            wscl[4 * b + j] = ws / np.float32(127.0)

    wqkv = np.stack(wq_all)
    wproj = np.stack(wp_all)
    wfc1 = np.stack(wf1_all)
    wfc2 = np.stack(wf2_all)

    posb = (np.asarray(inputs["pos_embed"][0], np.float32) +
            np.asarray(inputs["patch_b"], np.float32)[None, :]).astype(np.float32)
    patchWT = np.ascontiguousarray(np.asarray(inputs["patch_w"], np.float32).T)
    norm_w = np.asarray(inputs["norm_w"], np.float32)
    headWT = np.ascontiguousarray(np.asarray(inputs["head_w"], np.float32).T * norm_w[:, None])
    headb = np.asarray(inputs["head_b"], np.float32)[None, :]

    key = ("prog", depth)
    if key not in _CACHED:
        _CACHED[key] = build_program(depth)
    nc = _CACHED[key]

    in_maps = []
    for c in range(n_cores):
        imgs = slice(c * per, (c + 1) * per)
        xpT = np.ascontiguousarray(xp[imgs].reshape(per * hh * hh, CIN * p * p).T)
        in_maps.append(dict(
            xpT=xpT, posb=posb, patchWT=patchWT, headWT=headWT, headb=headb,
            wqkv=wqkv, wproj=wproj, wfc1=wfc1, wfc2=wfc2,
            mods=np.ascontiguousarray(
                np.broadcast_to(mods[:, :, None, imgs], (depth, 2, 128, per, 2, DIM))),
            wscl=wscl[None, :],
        ))

    return nc, in_maps


def _assemble(res, B=16, per=2):
    p = PATCH
    hh = IMG // p
    out = np.zeros((B, CIN, IMG, IMG), np.float32)
    for c in range(B // per):
        zo = res.results[c]["zout"]  # [512, 768]
        for i in range(per):
            zi = zo[i * 256:(i + 1) * 256]
            out[c * per + i] = zi.reshape(hh, hh, CIN, p, p).transpose(2, 0, 3, 1, 4).reshape(CIN, IMG, IMG)
    return out


def kernel(**inputs):
    nc, in_maps = _prepare(inputs)
    res = run_bass_kernel_spmd(nc, in_maps, list(range(len(in_maps))), trace=False)
    return _assemble(res)


# revision 33
# speedup vs baseline: 1.1100x; 1.0735x over previous
"""BitNet DiT on 8 Trainium2 NeuronCores — data-parallel over batch (2 images/core).

Host: patchify, time-embedding + adaLN modulation vectors, BitNet weight
quantization (ternary * per-tensor scale) -> fp8 upload.
Device: full 12-block DiT forward per core in a single Bass/Tile kernel.
BitNet matmuls run as exact integer arithmetic in bf16 (|values| <= 127,
fp32 accumulate). Attention runs via transposed-logits + ones-column
softmax-denominator trick.

v3 (vs v2):
- quant chains (ssq, rstd, amax, magic rounds) moved to DVE; rstd via
  Newton-Raphson rsqrt (bit trick + 2 iters) -> no Sqrt ACT table loads
  (only 2 table switches/block: exp <-> gelu).
- q/k computed weight-stationary producing [feat, tok] layout directly:
  kills 48 PE transposes + ACT evacs per block. Per-token activation
  scales applied via a broadcast c-row tile (PE column-transpose trick).
- attention softmax normalization batched: 6 heads per PSUM tile, one
  strided reciprocal + one broadcast multiply (was 48 ACT ops/block).
- softmax weights (eT) in bf16: halves SBUF + faster AV LDWEIGHTS.
- phase order tuned so PE never idles >3.4us (HAM stays at 2.4 GHz):
  v-matmuls before q/k, o-quant interleaved with attention tail.
"""
import math
import os
import sys
import numpy as np

sys.path.insert(0, "/opt/trn_rl_repo")

import ml_dtypes  # noqa: E402
import concourse.bass as bass  # noqa: E402
import concourse.mybir as mybir  # noqa: E402
import concourse.tile as tile  # noqa: E402
from concourse import bacc  # noqa: E402
from concourse.bass_utils import run_bass_kernel_spmd  # noqa: E402
from concourse.masks import make_identity  # noqa: E402

F32 = mybir.dt.float32
F32R = mybir.dt.float32r
I32 = mybir.dt.int32
FP8 = mybir.dt.float8e4
BF16 = mybir.dt.bfloat16
AX = mybir.AxisListType
OP = mybir.AluOpType
AF = mybir.ActivationFunctionType

DIM = 768
DEPTH = int(os.environ.get("KERNEL_DEPTH", "12"))
HEADS = 12
HD = 64
PATCH = 16
IMG = 256
CIN = 3
HID = 4 * DIM
EPS = 1e-6
P = 128
T = 512            # tokens per core (2 images x 256)
NT = T // P        # 4 token tiles
NTOK = 256         # tokens per image
KD = DIM // P      # 6
KH = HID // P      # 24
MAGIC = float(np.float32(3 * 2**22))  # 12582912.0 RNE round-to-int magic
RSQRT_SEED = 0x5F375A86

_CACHED = {}


def _mm_chunks(n):
    out = []
    s = 0
    while s < n:
        e = min(s + 512, n)
        out.append((s, e))
        s = e
    return out


def build_program(depth=DEPTH):
    nc = bacc.Bacc("TRN2", target_bir_lowering=False, debug=False, num_devices=8)

    xpT_d = nc.declare_dram_parameter("xpT", [DIM, T], F32, isOutput=False)
    posb_d = nc.declare_dram_parameter("posb", [NTOK, DIM], F32, isOutput=False)
    patchWT_d = nc.declare_dram_parameter("patchWT", [DIM, DIM], F32, isOutput=False)
    headWT_d = nc.declare_dram_parameter("headWT", [DIM, DIM], F32, isOutput=False)
    headb_d = nc.declare_dram_parameter("headb", [1, DIM], F32, isOutput=False)
    wqkv_d = nc.declare_dram_parameter("wqkv", [depth, DIM, 3 * DIM], FP8, isOutput=False)
    wproj_d = nc.declare_dram_parameter("wproj", [depth, DIM, DIM], FP8, isOutput=False)
    wfc1_d = nc.declare_dram_parameter("wfc1", [depth, DIM, HID], FP8, isOutput=False)
    wfc2_d = nc.declare_dram_parameter("wfc2", [depth, HID, DIM], FP8, isOutput=False)
    # modulation vectors: [block, norm(2), part, img(2), A/B(2), 768]
    mods_d = nc.declare_dram_parameter("mods", [depth, 2, P, 2, 2, DIM], F32, isOutput=False)
    wscl_d = nc.declare_dram_parameter("wscl", [1, 4 * depth], F32, isOutput=False)
    out_d = nc.declare_dram_parameter("zout", [T, DIM], F32, isOutput=True)

    with tile.TileContext(nc) as tc:
        from contextlib import ExitStack
        with ExitStack() as _ctx:
            constp = _ctx.enter_context(tc.tile_pool(name="const", bufs=1))
            residp = _ctx.enter_context(tc.tile_pool(name="resid", bufs=1))
            qkp = _ctx.enter_context(tc.tile_pool(name="qk", bufs=2))
            wp = _ctx.enter_context(tc.tile_pool(name="w", bufs=6))
            modp = _ctx.enter_context(tc.tile_pool(name="mod", bufs=2))
            tmp_ = _ctx.enter_context(tc.tile_pool(name="tm", bufs=2))
            gp = _ctx.enter_context(tc.tile_pool(name="g", bufs=3))
            hp = _ctx.enter_context(tc.tile_pool(name="h", bufs=2))
            xqtp = _ctx.enter_context(tc.tile_pool(name="xqt", bufs=2))
            xqop = _ctx.enter_context(tc.tile_pool(name="xqo", bufs=1))
            xq2p = _ctx.enter_context(tc.tile_pool(name="xq2", bufs=1))
            xqgp = _ctx.enter_context(tc.tile_pool(name="xqg", bufs=2))
            xqsp = _ctx.enter_context(tc.tile_pool(name="xqs", bufs=2))
            eTp = _ctx.enter_context(tc.tile_pool(name="eT", bufs=1))
            cbp = _ctx.enter_context(tc.tile_pool(name="cb", bufs=2))
            scp = _ctx.enter_context(tc.tile_pool(name="sc", bufs=64))
            ps_mm = _ctx.enter_context(tc.tile_pool(name="ps_mm", bufs=3, space="PSUM"))
            ps_lt = _ctx.enter_context(tc.tile_pool(name="ps_lt", bufs=2, space="PSUM"))
            ps_oa = _ctx.enter_context(tc.tile_pool(name="ps_oa", bufs=2, space="PSUM"))
            ps_x = _ctx.enter_context(tc.tile_pool(name="ps_x", bufs=1, space="PSUM"))

            idf = constp.tile([P, P], F32)
            make_identity(nc, idf[:])

            # broadcast w_scales/127 to all partitions
            wsrow = constp.tile([1, 4 * depth], F32)
            nc.sync.dma_start(wsrow[:], wscl_d[:])
            wsb = constp.tile([P, 4 * depth], F32)
            nc.gpsimd.partition_broadcast(wsb[:], wsrow[0:1, :])
            pmag = constp.tile([P, 1], F32)
            nc.vector.memset(pmag[:], MAGIC)

            z = residp.tile([P, NT, DIM], F32)
            v_aug = residp.tile([P, NT, HEADS, HD + 1], BF16)
            nc.vector.memset(v_aug[:, :, :, HD], 1.0)
            o_tm = residp.tile([P, NT, DIM], F32)

            # ---------- quant-chain helpers ----------
            # rsqrt seed for doubled input: rsqrt(2*hx) where hx = ms/2
            K2 = RSQRT_SEED - 0x400000

            def rstd_pair(ssq2):
                """[P,2] rstd = 1/sqrt(ssq/DIM+EPS), bit-trick + 2 Newton iters.
                All DVE; batched over a tile pair."""
                hx = scp.tile([P, 2], F32, tag="sc", name="hx")
                nc.vector.tensor_scalar(hx[:], ssq2, 0.5 / DIM, EPS / 2,
                                        OP.mult, OP.add)
                yi = scp.tile([P, 2], I32, tag="sc", name="yi")
                nc.vector.tensor_scalar(yi[:], hx[:].bitcast(I32), 1, None,
                                        OP.arith_shift_right)
                yn = scp.tile([P, 2], I32, tag="sc", name="yn")
                nc.vector.tensor_scalar(yn[:], yi[:], -1, None, OP.bitwise_xor)
                y = scp.tile([P, 2], I32, tag="sc", name="y0")
                nc.vector.tensor_scalar(y[:], yn[:], K2 + 1, None, OP.add)
                yf = y[:].bitcast(F32)
                for it in range(2):
                    a = scp.tile([P, 2], F32, tag="sc", name=f"nra{it}")
                    nc.vector.tensor_tensor(a[:], yf, yf, OP.mult)
                    b = scp.tile([P, 2], F32, tag="sc", name=f"nrb{it}")
                    nc.vector.tensor_tensor(b[:], a[:], hx[:], OP.mult)
                    c = scp.tile([P, 2], F32, tag="sc", name=f"nrc{it}")
                    nc.vector.tensor_scalar(c[:], b[:], -1.0, 1.5, OP.mult, OP.add)
                    y2 = scp.tile([P, 2], F32, tag="sc", name=f"nry{it}")
                    nc.vector.tensor_tensor(y2[:], yf, c[:], OP.mult)
                    yf = y2[:]
                return yf

            sq_scr = tmp_.tile([P, DIM], F32, tag="tm", name="sqscr")

            def ssq_act(src_ap, sv_col):
                """sum(src^2) per row on ACT (Square table-free), accum into
                the given [P,1] column."""
                nc.scalar.activation(sq_scr[:], src_ap, AF.Square,
                                     accum_out=sv_col)

            def norm_mod(t, mt, rstd_ap, dst, add_eng):
                """dst = (z[t]*rstd) * modA + modB  (STT on DVE, add on
                add_eng for load balancing)."""
                img = t // 2
                nc.vector.scalar_tensor_tensor(dst, z[:, t, :], rstd_ap,
                                               mt[:, img, 0, :], OP.mult, OP.mult)
                add_eng.tensor_tensor(dst, dst, mt[:, img, 1, :], OP.add)

            def quant_pair(src_aps, ws_idx):
                """amax over each of 2 sources -> batched (s127[P,2], c[P,2])."""
                amax = scp.tile([P, 2], F32, tag="sc", name="amax")
                for j, src in enumerate(src_aps):
                    nc.vector.tensor_reduce(amax[:, j:j + 1], src, axis=AX.X,
                                            op=OP.max, apply_absolute_value=True)
                rs = scp.tile([P, 2], F32, tag="sc", name="rcp")
                nc.vector.reciprocal(rs[:], amax[:])
                s127 = scp.tile([P, 2], F32, tag="sc", name="s127")
                nc.vector.tensor_scalar_mul(s127[:], rs[:], 127.0)
                c = scp.tile([P, 2], F32, tag="sc", name="cc")
                nc.vector.tensor_scalar(c[:], amax[:], wsb[:, ws_idx:ws_idx + 1],
                                        None, OP.mult)
                return s127, c

            def round_dma_act(src_ap, s127_col, dst_slice):
                """magic-round on ACT (in-place), unmagic to bf16 on DVE,
                DMA-transpose into dst."""
                nc.scalar.activation(src_ap, src_ap, AF.Identity,
                                     scale=s127_col, bias=pmag[:])
                xq = xqsp.tile([P, DIM], BF16, tag="xqs", name="xqs")
                nc.vector.tensor_scalar(xq[:], src_ap, MAGIC, None, OP.subtract)
                nc.sync.dma_start_transpose(dst_slice, xq[:])

            def round_dma_dve(src_ap, s127_col, dst_slice):
                """magic-round + unmagic both on DVE (for attn phase where
                ACT is busy with exp)."""
                nc.vector.tensor_scalar(src_ap, src_ap, s127_col, MAGIC,
                                        OP.mult, OP.add)
                xq = xqsp.tile([P, DIM], BF16, tag="xqs", name="xqs")
                nc.vector.tensor_scalar(xq[:], src_ap, MAGIC, None, OP.subtract)
                nc.sync.dma_start_transpose(dst_slice, xq[:])

            # ---------------- patch embed ----------------
            posb_sb = wp.tile([P, 2, DIM], F32, tag="w", name="posb_sb")
            nc.sync.dma_start(posb_sb[:], posb_d.rearrange("(a p) d -> p a d", p=P))
            xpT = qkp.tile([P, KD, T], F32, tag="qk")
            nc.sync.dma_start(xpT[:], xpT_d.rearrange("(o p) t -> p o t", p=P))
            pw_pieces = []
            for i in range(3):
                pwp = wp.tile([P, 2, DIM], F32, tag="w", name="pwp")
                nc.gpsimd.dma_start(
                    pwp[:], patchWT_d[i * 2 * P:(i + 1) * 2 * P, :].rearrange(
                        "(o p) d -> p o d", p=P))
                pw_pieces.append(pwp)
            for t in range(NT):
                for (cs, ce) in _mm_chunks(DIM):
                    pt = ps_mm.tile([P, 512], F32, tag="mm", name="pmm")[:, : ce - cs]
                    for k in range(KD):
                        nc.tensor.matmul(pt[:], xpT[:, k, t * P:(t + 1) * P],
                                         pw_pieces[k // 2][:, k % 2, cs:ce],
                                         start=(k == 0), stop=(k == KD - 1))
                    nc.vector.tensor_tensor(z[:, t, cs:ce], pt[:], posb_sb[:, t % 2, cs:ce], OP.add)

            def load_w(dram, b, kchunks, width, npieces):
                """Stage one linear's transposed fp8 weights as npieces tiles."""
                span = kchunks // npieces
                tiles = []
                for i in range(npieces):
                    wt = wp.tile([P, span, width], FP8, tag="w")
                    nc.gpsimd.dma_start(
                        wt[:],
                        dram[b, i * span * P:(i + 1) * span * P, :].rearrange(
                            "(o p) f -> p o f", p=P))
                    tiles.append(wt)
                return tiles, span

            def load_mods(b_, n_, name):
                mt = modp.tile([P, 2, 2, DIM], F32, tag="mod", name=name)
                nc.gpsimd.dma_start(mt[:], mods_d[b_, n_])
                return mt

            def p1_pair(pi, mt, ws_idx, xqT_dst, c_out, eng):
                """norm1/norm2 + quant for tile pair (2pi, 2pi+1)."""
                ssq2 = scp.tile([P, 2], F32, tag="sc", name="ssq2")
                for j in range(2):
                    ssq_act(z[:, 2 * pi + j, :], ssq2[:, j:j + 1])
                rst2 = rstd_pair(ssq2[:])
                hs = []
                for j in range(2):
                    h = hp.tile([P, DIM], F32, tag="h")
                    norm_mod(2 * pi + j, mt, rst2[:, j:j + 1], h[:], eng)
                    hs.append(h)
                s127, cpr = quant_pair([h[:] for h in hs], ws_idx)
                for j in range(2):
                    t = 2 * pi + j
                    c_out[t] = cpr[:, j:j + 1]
                    round_dma_act(hs[j][:], s127[:, j:j + 1],
                                  xqT_dst[:, :, t * P:(t + 1) * P])

            # ---- prologue: phase 1 of block 0 ----
            mt1_nxt = load_mods(0, 0, "mt1")
            xqT_cur = xqtp.tile([P, KD, T], BF16, tag="xqt")
            c_cur = [None] * NT
            for pi in range(2):
                p1_pair(pi, mt1_nxt, 0, xqT_cur, c_cur, nc.vector)

            for b in range(depth):
                xqT, c_list = xqT_cur, c_cur
                mt2 = load_mods(b, 1, "mt2")
                wq_tiles, wq_span = load_w(wqkv_d, b, KD, 3 * DIM, 3)

                with nc.named_scope(f"b{b}_qkv"):
                    q_fm = qkp.tile([P, KD, T], BF16, tag="qk", name="q_fm")
                    k_fm = qkp.tile([P, KD, T], BF16, tag="qk", name="k_fm")
                    cb = cbp.tile([P, T], F32, tag="cb", name="cbb")

                    def build_cb(half):
                        # per-token scales -> [1,256] row via PE transpose,
                        # then broadcast to all partitions.
                        crow_ps = ps_x.tile([1, 2, P], F32, tag="x", name="crow_ps")
                        for j in range(2):
                            nc.tensor.transpose(crow_ps[:, j, :],
                                                c_list[2 * half + j], idf[:])
                        crow = cbp.tile([1, NTOK], F32, tag="cb", name="crow")
                        nc.vector.tensor_copy(
                            crow[:], crow_ps[:].rearrange("a b c -> a (b c)"))
                        nc.gpsimd.partition_broadcast(
                            cb[:, half * NTOK:(half + 1) * NTOK], crow[0:1, :])

                    def v_mm(t):
                        for (cs, ce) in _mm_chunks(DIM):
                            pt = ps_mm.tile([P, 512], F32, tag="mm", name="pmm")[:, : ce - cs]
                            for k in range(KD):
                                wt = wq_tiles[k // wq_span]
                                nc.tensor.matmul(
                                    pt[:], xqT[:, k, t * P:(t + 1) * P],
                                    wt[:, k % wq_span, 2 * DIM + cs:2 * DIM + ce],
                                    start=(k == 0), stop=(k == KD - 1))
                            nc.scalar.activation(
                                v_aug[:, t, cs // HD:ce // HD, 0:HD], pt[:],
                                AF.Identity, scale=c_list[t])

                    def qk_mm(half):
                        # weight-stationary, psum [feat 128, tok 256]
                        ts0 = half * NTOK
                        for fc in range(12):
                            pt = ps_mm.tile([P, 512], F32, tag="mm",
                                            name="pmm")[:, :NTOK]
                            for k in range(KD):
                                wt = wq_tiles[k // wq_span]
                                nc.tensor.matmul(
                                    pt[:], wt[:, k % wq_span, fc * P:(fc + 1) * P],
                                    xqT[:, k, ts0:ts0 + NTOK],
                                    start=(k == 0), stop=(k == KD - 1))
                            if fc < 6:
                                nc.vector.scalar_tensor_tensor(
                                    q_fm[:, fc, ts0:ts0 + NTOK], pt[:], 0.125,
                                    cb[:, ts0:ts0 + NTOK], OP.mult, OP.mult)
                            else:
                                nc.vector.tensor_tensor(
                                    k_fm[:, fc - 6, ts0:ts0 + NTOK], pt[:],
                                    cb[:, ts0:ts0 + NTOK], OP.mult)

                    build_cb(0)
                    v_mm(0)
                    v_mm(1)
                    qk_mm(0)
                    build_cb(1)
                    v_mm(2)
                    v_mm(3)
                    qk_mm(1)

                # --- attention ---
                wp_tiles, wp_span = load_w(wproj_d, b, KD, DIM, 2)
                xqoT = xqop.tile([P, KD, T], BF16, tag="xqo")
                co_s = [None] * NT

                def o_quant_pair(img):
                    srcs = [o_tm[:, 2 * img + j, :] for j in range(2)]
                    s127, cpr = quant_pair(srcs, 4 * b + 1)
                    for j in range(2):
                        t = 2 * img + j
                        co_s[t] = cpr[:, j:j + 1]
                        round_dma_dve(srcs[j], s127[:, j:j + 1],
                                      xqoT[:, :, t * P:(t + 1) * P])

                with nc.named_scope(f"b{b}_attn"):
                    for img in range(2):
                        for g in range(2):
                            eTg = eTp.tile([P, 6, 2, NTOK], BF16, tag="eT")
                            for h6 in range(6):
                                hh = 6 * g + h6
                                po = (hh % 2) * HD
                                ch = hh // 2
                                lt = ps_lt.tile([P, 2, NTOK], F32, tag="lt")
                                for mt in range(2):
                                    nc.tensor.matmul(
                                        lt[:, mt, :],
                                        k_fm[po:po + HD, ch,
                                             img * NTOK + mt * P: img * NTOK + (mt + 1) * P],
                                        q_fm[po:po + HD, ch,
                                             img * NTOK: (img + 1) * NTOK],
                                        start=True, stop=True)
                                nc.scalar.activation(eTg[:, h6], lt[:], AF.Exp)
                            for nt in range(2):
                                oa = ps_oa.tile([P, 6, HD + 1], F32, tag="oa")
                                for h6 in range(6):
                                    for mt in range(2):
                                        nc.tensor.matmul(
                                            oa[:, h6, :],
                                            eTg[:, h6, mt, nt * P:(nt + 1) * P],
                                            v_aug[:, img * 2 + mt, 6 * g + h6, :],
                                            start=(mt == 0), stop=(mt == 1))
                                rinv = scp.tile([P, 6], F32, tag="sc", name="rinv")
                                nc.vector.reciprocal(rinv[:], oa[:, :, HD])
                                dst = o_tm[:, img * 2 + nt,
                                           384 * g:384 * (g + 1)].rearrange(
                                               "p (h d) -> p h d", h=6)
                                nc.vector.tensor_tensor(
                                    dst, oa[:, :, 0:HD],
                                    rinv[:, :, None].broadcast_to([P, 6, HD]),
                                    OP.mult)
                            if g == 1:
                                o_quant_pair(img)

                # --- proj + n2 chain ---
                wf1_tiles, wf1_span = load_w(wfc1_d, b, KD, HID, 3)
                xq2T = xq2p.tile([P, KD, T], BF16, tag="xq2")
                c3s = [None] * NT

                with nc.named_scope(f"b{b}_proj"):
                    for t in range(NT):
                        for (cs, ce) in _mm_chunks(DIM):
                            pt = ps_mm.tile([P, 512], F32, tag="mm", name="pmm")[:, : ce - cs]
                            for k in range(KD):
                                wt = wp_tiles[k // wp_span]
                                nc.tensor.matmul(pt[:], xqoT[:, k, t * P:(t + 1) * P],
                                                 wt[:, k % wp_span, cs:ce],
                                                 start=(k == 0), stop=(k == KD - 1))
                            nc.vector.scalar_tensor_tensor(
                                z[:, t, cs:ce], pt[:], co_s[t], z[:, t, cs:ce],
                                OP.mult, OP.add)
                        if t % 2 == 1:
                            p1_pair(t // 2, mt2, 4 * b + 2, xq2T, c3s, nc.vector)

                # --- fc1 + gelu + g-quant ---
                wf2_tiles, wf2_span = load_w(wfc2_d, b, KH, DIM, 3)
                xqg = [None] * NT
                c4s = [None] * NT
                gs = [None] * NT

                def gquant(t):
                    gh0, gh1 = gs[t]
                    am = scp.tile([P, 2], F32, tag="sc", name="amg")
                    nc.vector.tensor_reduce(am[:, 0:1], gh0[:], axis=AX.X, op=OP.max,
                                            apply_absolute_value=True)
                    nc.vector.tensor_reduce(am[:, 1:2], gh1[:], axis=AX.X, op=OP.max,
                                            apply_absolute_value=True)
                    ac = scp.tile([P, 1], F32, tag="sc", name="amaxc")
                    nc.vector.tensor_tensor(ac[:], am[:, 0:1], am[:, 1:2], OP.max)
                    rs = scp.tile([P, 1], F32, tag="sc", name="rcp")
                    nc.vector.reciprocal(rs[:], ac[:])
                    s127 = scp.tile([P, 1], F32, tag="sc", name="s127")
                    nc.vector.tensor_scalar_mul(s127[:], rs[:], 127.0)
                    c = scp.tile([P, 1], F32, tag="sc", name="cc")
                    nc.vector.tensor_scalar(c[:], ac[:], wsb[:, 4 * b + 3:4 * b + 4],
                                            None, OP.mult)
                    c4s[t] = c
                    xg = xqgp.tile([P, KH, P], BF16, tag="xqg")
                    xqg[t] = xg
                    for i, gh in enumerate((gh0, gh1)):
                        # magic on ACT (Identity, no table switch), unmagic DVE
                        nc.scalar.activation(gh[:], gh[:], AF.Identity,
                                             scale=s127[:], bias=pmag[:])
                        xq = xqsp.tile([P, HID // 2], BF16, tag="xqs", name="xq24s")
                        nc.vector.tensor_scalar(xq[:], gh[:], MAGIC, None, OP.subtract)
                        nc.sync.dma_start_transpose(xg[:, i * 12:(i + 1) * 12, :], xq[:])

                # --- fc1/fc2 interleaved per tile, fc2 fused with next p1 ---
                fuse = b + 1 < depth
                if fuse:
                    mt1_nxt = load_mods(b + 1, 0, "mt1")
                    xqT_cur = xqtp.tile([P, KD, T], BF16, tag="xqt")
                    c_cur = [None] * NT

                def fc1_t(t):
                    gh0 = gp.tile([P, HID // 2], F32, tag="g")
                    gh1 = gp.tile([P, HID // 2], F32, tag="g")
                    gs[t] = (gh0, gh1)
                    for ci, (cs, ce) in enumerate(_mm_chunks(HID)):
                        pt = ps_mm.tile([P, 512], F32, tag="mm", name="pmm")[:, : ce - cs]
                        for k in range(KD):
                            wt = wf1_tiles[k // wf1_span]
                            nc.tensor.matmul(pt[:], xq2T[:, k, t * P:(t + 1) * P],
                                             wt[:, k % wf1_span, cs:ce],
                                             start=(k == 0), stop=(k == KD - 1))
                        gh = gh0 if ci < 3 else gh1
                        off = cs - (0 if ci < 3 else HID // 2)
                        nc.scalar.activation(gh[:, off:off + ce - cs], pt[:],
                                             AF.Gelu_apprx_tanh, scale=c3s[t][:])

                def fc2_t(t):
                    for (cs, ce) in _mm_chunks(DIM):
                        pt = ps_mm.tile([P, 512], F32, tag="mm", name="pmm")[:, : ce - cs]
                        for k in range(KH):
                            wt = wf2_tiles[k // wf2_span]
                            nc.tensor.matmul(pt[:], xqg[t][:, k, :],
                                             wt[:, k % wf2_span, cs:ce],
                                             start=(k == 0), stop=(k == KH - 1))
                        nc.vector.scalar_tensor_tensor(
                            z[:, t, cs:ce], pt[:], c4s[t][:], z[:, t, cs:ce],
                            OP.mult, OP.add)

                with nc.named_scope(f"b{b}_mlp"):
                    for t in range(NT):
                        fc1_t(t)
                        if t > 0:
                            gquant(t - 1)
                            fc2_t(t - 1)
                        if t == 2 and fuse:
                            p1_pair(0, mt1_nxt, 4 * (b + 1), xqT_cur, c_cur,
                                    nc.gpsimd)
                    gquant(NT - 1)
                    fc2_t(NT - 1)
                    if fuse:
                        p1_pair(1, mt1_nxt, 4 * (b + 1), xqT_cur, c_cur,
                                nc.gpsimd)

            # ---------------- final norm + head ----------------
            with nc.named_scope("head"):
                hw_pieces = []
                for i in range(3):
                    hwp = wp.tile([P, 2, DIM], F32, tag="w", name="hwp")
                    nc.gpsimd.dma_start(
                        hwp[:], headWT_d[i * 2 * P:(i + 1) * 2 * P, :].rearrange(
                            "(o p) d -> p o d", p=P))
                    hw_pieces.append(hwp)
                hbrow = tmp_.tile([1, DIM], F32, tag="tm", name="hbrow")
                nc.sync.dma_start(hbrow[:], headb_d[:])
                hbb = wp.tile([P, DIM], F32, tag="w", name="hbb")
                nc.gpsimd.partition_broadcast(hbb[:], hbrow[0:1, :])
                rst_cols = []
                for pi in range(2):
                    ssq2 = scp.tile([P, 2], F32, tag="sc", name="ssqh")
                    for j in range(2):
                        ssq_act(z[:, 2 * pi + j, :], ssq2[:, j:j + 1])
                    rst2 = rstd_pair(ssq2[:])
                    rst_cols += [rst2[:, 0:1], rst2[:, 1:2]]
                for t in range(NT):
                    zn = hp.tile([P, DIM], F32, tag="h")
                    nc.vector.tensor_scalar_mul(zn[:], z[:, t, :], rst_cols[t])
                    znT = hp.tile([P, DIM], F32, tag="h")
                    for g0 in range(0, KD, 4):
                        gn = min(4, KD - g0)
                        ptb = ps_lt.tile([P, 512], F32, tag="lt", name="ptb")[:, : gn * P]
                        for j in range(gn):
                            nc.tensor.transpose(ptb[:, j * P:(j + 1) * P],
                                                zn[:, (g0 + j) * P:(g0 + j + 1) * P], idf[:])
                        nc.vector.tensor_copy(znT[:, g0 * P:(g0 + gn) * P], ptb[:])
                    for (cs, ce) in _mm_chunks(DIM):
                        pt = ps_mm.tile([P, 512], F32, tag="mm", name="pmm")[:, : ce - cs]
                        for k in range(KD):
                            nc.tensor.matmul(pt[:], znT[:, k * P:(k + 1) * P],
                                             hw_pieces[k // 2][:, k % 2, cs:ce],
                                             start=(k == 0), stop=(k == KD - 1))
                        ot = tmp_.tile([P, DIM], F32, tag="tm", name="ot")[:, : ce - cs]
                        nc.vector.tensor_tensor(ot[:], pt[:], hbb[:, cs:ce], OP.add)
                        nc.sync.dma_start(out_d[t * P:(t + 1) * P, cs:ce], ot[:])

    nc.compile()
    return nc


# ---------------------------------------------------------------------------
# host-side numerics (numpy, fp32 — matches jax CPU within ~1e-7)

def _gelu_tanh(x):
    x = x.astype(np.float32)
    c = np.float32(math.sqrt(2.0 / math.pi))
    return np.float32(0.5) * x * (np.float32(1.0) +
                                  np.tanh(c * (x + np.float32(0.044715) * x * x * x)))


def _time_embedding(t, t_w1, t_b1, t_w2, t_b2):
    half = DIM // 2
    freqs = np.exp(-np.log(10000.0) * np.arange(half, dtype=np.float32) / (half - 1)).astype(np.float32)
    args = t[:, None].astype(np.float32) * freqs[None, :]
    emb = np.concatenate([np.sin(args), np.cos(args)], axis=-1).astype(np.float32)
    h = _gelu_tanh(emb @ t_w1.T + t_b1)
    return (h @ t_w2.T + t_b2).astype(np.float32)


def _quant_w(w):
    ws = np.float32(np.mean(np.abs(w), dtype=np.float64)) + np.float32(1e-5)
    wq = np.clip(np.round(w.astype(np.float32) / ws), -1.0, 1.0)
    return wq, ws


def _prepare(inputs):
    x = np.asarray(inputs["x"], np.float32)
    t = np.asarray(inputs["t"], np.float32)
    B = x.shape[0]
    n_cores = 8
    per = B // n_cores  # 2
    p = PATCH
    hh = IMG // p

    xp = x.reshape(B, CIN, hh, p, hh, p).transpose(0, 2, 4, 1, 3, 5).reshape(B, hh * hh, CIN * p * p)

    t_emb = _time_embedding(t, inputs["t_w1"], inputs["t_b1"], inputs["t_w2"], inputs["t_b2"])
    silu = (t_emb / (1.0 + np.exp(-t_emb))).astype(np.float32)

    depth = DEPTH
    mods = np.zeros((depth, 2, B, 2, DIM), np.float32)  # [blk, norm, img, A/B, D]
    wscl = np.zeros((4 * depth,), np.float32)
    wq_all, wp_all, wf1_all, wf2_all = [], [], [], []
    for b in range(depth):
        mod = silu @ np.asarray(inputs["blk_ada_w"][b], np.float32).T + np.asarray(
            inputs["blk_ada_b"][b], np.float32)
        sh1, sc1, sh2, sc2 = np.split(mod, 4, axis=-1)
        n1 = np.asarray(inputs["blk_norm1"][b], np.float32)
        n2 = np.asarray(inputs["blk_norm2"][b], np.float32)
        mods[b, 0, :, 0, :] = n1[None, :] * (1.0 + sc1)
        mods[b, 0, :, 1, :] = sh1
        mods[b, 1, :, 0, :] = n2[None, :] * (1.0 + sc2)
        mods[b, 1, :, 1, :] = sh2

        for j, (nm, lst) in enumerate([("blk_qkv", wq_all), ("blk_proj", wp_all),
                                       ("blk_fc1", wf1_all), ("blk_fc2", wf2_all)]):
            wq, ws = _quant_w(np.asarray(inputs[nm][b], np.float32))
            lst.append(np.ascontiguousarray(wq.T).astype(ml_dtypes.float8_e4m3))
            wscl[4 * b + j] = ws / np.float32(127.0)

    wqkv = np.stack(wq_all)
    wproj = np.stack(wp_all)
    wfc1 = np.stack(wf1_all)
    wfc2 = np.stack(wf2_all)

    posb = (np.asarray(inputs["pos_embed"][0], np.float32) +
            np.asarray(inputs["patch_b"], np.float32)[None, :]).astype(np.float32)
    patchWT = np.ascontiguousarray(np.asarray(inputs["patch_w"], np.float32).T)
    norm_w = np.asarray(inputs["norm_w"], np.float32)
    headWT = np.ascontiguousarray(np.asarray(inputs["head_w"], np.float32).T * norm_w[:, None])
    headb = np.asarray(inputs["head_b"], np.float32)[None, :]

    key = ("prog", depth)
    if key not in _CACHED:
        _CACHED[key] = build_program(depth)
    nc = _CACHED[key]

    in_maps = []
    for c in range(n_cores):
        imgs = slice(c * per, (c + 1) * per)
        xpT = np.ascontiguousarray(xp[imgs].reshape(per * hh * hh, CIN * p * p).T)
        in_maps.append(dict(
            xpT=xpT, posb=posb, patchWT=patchWT, headWT=headWT, headb=headb,
            wqkv=wqkv, wproj=wproj, wfc1=wfc1, wfc2=wfc2,
            mods=np.ascontiguousarray(
                np.broadcast_to(mods[:, :, None, imgs], (depth, 2, 128, per, 2, DIM))),
            wscl=wscl[None, :],
        ))

    return nc, in_maps


def _assemble(res, B=16, per=2):
    p = PATCH
    hh = IMG // p
    out = np.zeros((B, CIN, IMG, IMG), np.float32)
    for c in range(B // per):
        zo = res.results[c]["zout"]  # [512, 768]
        for i in range(per):
            zi = zo[i * 256:(i + 1) * 256]
            out[c * per + i] = zi.reshape(hh, hh, CIN, p, p).transpose(2, 0, 3, 1, 4).reshape(CIN, IMG, IMG)
    return out


def kernel(**inputs):
    nc, in_maps = _prepare(inputs)
    res = run_bass_kernel_spmd(nc, in_maps, list(range(len(in_maps))), trace=False)
    return _assemble(res)


# revision 36
# speedup vs baseline: 1.1176x; 1.0068x over previous
"""BitNet DiT on 8 Trainium2 NeuronCores — data-parallel over batch (2 images/core).

Host: patchify, time-embedding + adaLN modulation vectors, BitNet weight
quantization (ternary * per-tensor scale) -> fp8 upload.
Device: full 12-block DiT forward per core in a single Bass/Tile kernel.
BitNet matmuls run as exact integer arithmetic in bf16 (|values| <= 127,
fp32 accumulate). Attention runs via transposed-logits + ones-column
softmax-denominator trick.

v3 (vs v2):
- quant chains (ssq, rstd, amax, magic rounds) moved to DVE; rstd via
  Newton-Raphson rsqrt (bit trick + 2 iters) -> no Sqrt ACT table loads
  (only 2 table switches/block: exp <-> gelu).
- q/k computed weight-stationary producing [feat, tok] layout directly:
  kills 48 PE transposes + ACT evacs per block. Per-token activation
  scales applied via a broadcast c-row tile (PE column-transpose trick).
- attention softmax normalization batched: 6 heads per PSUM tile, one
  strided reciprocal + one broadcast multiply (was 48 ACT ops/block).
- softmax weights (eT) in bf16: halves SBUF + faster AV LDWEIGHTS.
- phase order tuned so PE never idles >3.4us (HAM stays at 2.4 GHz):
  v-matmuls before q/k, o-quant interleaved with attention tail.
"""
import math
import os
import sys
import numpy as np

sys.path.insert(0, "/opt/trn_rl_repo")

import ml_dtypes  # noqa: E402
import concourse.bass as bass  # noqa: E402
import concourse.mybir as mybir  # noqa: E402
import concourse.tile as tile  # noqa: E402
from concourse import bacc  # noqa: E402
from concourse.bass_utils import run_bass_kernel_spmd  # noqa: E402
from concourse.masks import make_identity  # noqa: E402

F32 = mybir.dt.float32
F32R = mybir.dt.float32r
I32 = mybir.dt.int32
FP8 = mybir.dt.float8e4
BF16 = mybir.dt.bfloat16
AX = mybir.AxisListType
OP = mybir.AluOpType
AF = mybir.ActivationFunctionType

DIM = 768
DEPTH = int(os.environ.get("KERNEL_DEPTH", "12"))
HEADS = 12
HD = 64
PATCH = 16
IMG = 256
CIN = 3
HID = 4 * DIM
EPS = 1e-6
P = 128
T = 512            # tokens per core (2 images x 256)
NT = T // P        # 4 token tiles
NTOK = 256         # tokens per image
KD = DIM // P      # 6
KH = HID // P      # 24
MAGIC = float(np.float32(3 * 2**22))  # 12582912.0 RNE round-to-int magic
RSQRT_SEED = 0x5F375A86

_CACHED = {}


def _mm_chunks(n):
    out = []
    s = 0
    while s < n:
        e = min(s + 512, n)
        out.append((s, e))
        s = e
    return out


def build_program(depth=DEPTH):
    nc = bacc.Bacc("TRN2", target_bir_lowering=False, debug=False, num_devices=8)

    xpT_d = nc.declare_dram_parameter("xpT", [DIM, T], F32, isOutput=False)
    posb_d = nc.declare_dram_parameter("posb", [NTOK, DIM], F32, isOutput=False)
    patchWT_d = nc.declare_dram_parameter("patchWT", [DIM, DIM], F32, isOutput=False)
    headWT_d = nc.declare_dram_parameter("headWT", [DIM, DIM], F32, isOutput=False)
    headb_d = nc.declare_dram_parameter("headb", [1, DIM], F32, isOutput=False)
    wqkv_d = nc.declare_dram_parameter("wqkv", [depth, DIM, 3 * DIM], FP8, isOutput=False)
    wproj_d = nc.declare_dram_parameter("wproj", [depth, DIM, DIM], FP8, isOutput=False)
    wfc1_d = nc.declare_dram_parameter("wfc1", [depth, DIM, HID], FP8, isOutput=False)
    wfc2_d = nc.declare_dram_parameter("wfc2", [depth, HID, DIM], FP8, isOutput=False)
    # modulation vectors: [block, norm(2), part, img(2), A/B(2), 768]
    mods_d = nc.declare_dram_parameter("mods", [depth, 2, P, 2, 2, DIM], F32, isOutput=False)
    wscl_d = nc.declare_dram_parameter("wscl", [1, 4 * depth], F32, isOutput=False)
    out_d = nc.declare_dram_parameter("zout", [T, DIM], F32, isOutput=True)

    with tile.TileContext(nc) as tc:
        from contextlib import ExitStack
        with ExitStack() as _ctx:
            constp = _ctx.enter_context(tc.tile_pool(name="const", bufs=1))
            residp = _ctx.enter_context(tc.tile_pool(name="resid", bufs=1))
            qkp = _ctx.enter_context(tc.tile_pool(name="qk", bufs=2))
            wp = _ctx.enter_context(tc.tile_pool(name="w", bufs=6))
            modp = _ctx.enter_context(tc.tile_pool(name="mod", bufs=2))
            tmp_ = _ctx.enter_context(tc.tile_pool(name="tm", bufs=2))
            gp = _ctx.enter_context(tc.tile_pool(name="g", bufs=3))
            hp = _ctx.enter_context(tc.tile_pool(name="h", bufs=4))
            xqtp = _ctx.enter_context(tc.tile_pool(name="xqt", bufs=4))
            xqop = _ctx.enter_context(tc.tile_pool(name="xqo", bufs=2))
            xq2p = _ctx.enter_context(tc.tile_pool(name="xq2", bufs=2))
            xqgp = _ctx.enter_context(tc.tile_pool(name="xqg", bufs=2))
            xqsp = _ctx.enter_context(tc.tile_pool(name="xqs", bufs=3))
            eTp = _ctx.enter_context(tc.tile_pool(name="eT", bufs=1))
            cbp = _ctx.enter_context(tc.tile_pool(name="cb", bufs=3))
            scp = _ctx.enter_context(tc.tile_pool(name="sc", bufs=64))
            ps_mm = _ctx.enter_context(tc.tile_pool(name="ps_mm", bufs=3, space="PSUM"))
            ps_lt = _ctx.enter_context(tc.tile_pool(name="ps_lt", bufs=2, space="PSUM"))
            ps_oa = _ctx.enter_context(tc.tile_pool(name="ps_oa", bufs=2, space="PSUM"))
            ps_x = _ctx.enter_context(tc.tile_pool(name="ps_x", bufs=1, space="PSUM"))

            idf = constp.tile([P, P], F32)
            make_identity(nc, idf[:])

            # broadcast w_scales/127 to all partitions
            wsrow = constp.tile([1, 4 * depth], F32)
            nc.sync.dma_start(wsrow[:], wscl_d[:])
            wsb = constp.tile([P, 4 * depth], F32)
            nc.gpsimd.partition_broadcast(wsb[:], wsrow[0:1, :])
            pmag = constp.tile([P, 1], F32)
            nc.vector.memset(pmag[:], MAGIC)

            z = residp.tile([P, NT, DIM], F32)
            v_aug = residp.tile([P, NT, HEADS, HD + 1], BF16)
            nc.vector.memset(v_aug[:, :, :, HD], 1.0)
            o_tm = residp.tile([P, NT, DIM], F32)

            # ---------- quant-chain helpers ----------
            # rsqrt seed for doubled input: rsqrt(2*hx) where hx = ms/2
            K2 = RSQRT_SEED - 0x400000

            def rstd_pair(ssq2):
                """[P,2] rstd = 1/sqrt(ssq/DIM+EPS), bit-trick + 2 Newton iters.
                All DVE; batched over a tile pair."""
                hx = scp.tile([P, 2], F32, tag="sc", name="hx")
                nc.vector.tensor_scalar(hx[:], ssq2, 0.5 / DIM, EPS / 2,
                                        OP.mult, OP.add)
                yi = scp.tile([P, 2], I32, tag="sc", name="yi")
                nc.vector.tensor_scalar(yi[:], hx[:].bitcast(I32), 1, None,
                                        OP.arith_shift_right)
                yn = scp.tile([P, 2], I32, tag="sc", name="yn")
                nc.vector.tensor_scalar(yn[:], yi[:], -1, None, OP.bitwise_xor)
                y = scp.tile([P, 2], I32, tag="sc", name="y0")
                nc.vector.tensor_scalar(y[:], yn[:], K2 + 1, None, OP.add)
                yf = y[:].bitcast(F32)
                for it in range(2):
                    a = scp.tile([P, 2], F32, tag="sc", name=f"nra{it}")
                    nc.vector.tensor_tensor(a[:], yf, yf, OP.mult)
                    b = scp.tile([P, 2], F32, tag="sc", name=f"nrb{it}")
                    nc.vector.tensor_tensor(b[:], a[:], hx[:], OP.mult)
                    c = scp.tile([P, 2], F32, tag="sc", name=f"nrc{it}")
                    nc.vector.tensor_scalar(c[:], b[:], -1.0, 1.5, OP.mult, OP.add)
                    y2 = scp.tile([P, 2], F32, tag="sc", name=f"nry{it}")
                    nc.vector.tensor_tensor(y2[:], yf, c[:], OP.mult)
                    yf = y2[:]
                return yf

            sq_scr = tmp_.tile([P, DIM], F32, tag="tm", name="sqscr")

            def ssq_act(src_ap, sv_col):
                """sum(src^2) per row on ACT (Square table-free), accum into
                the given [P,1] column."""
                nc.scalar.activation(sq_scr[:], src_ap, AF.Square,
                                     accum_out=sv_col)

            def norm_mod(t, mt, rstd_ap, dst, add_eng):
                """dst = (z[t]*rstd) * modA + modB  (STT on DVE, add on
                add_eng for load balancing)."""
                img = t // 2
                nc.vector.scalar_tensor_tensor(dst, z[:, t, :], rstd_ap,
                                               mt[:, img, 0, :], OP.mult, OP.mult)
                add_eng.tensor_tensor(dst, dst, mt[:, img, 1, :], OP.add)

            def quant_pair(src_aps, ws_idx):
                """amax over each of 2 sources -> batched (s127[P,2], c[P,2])."""
                amax = scp.tile([P, 2], F32, tag="sc", name="amax")
                for j, src in enumerate(src_aps):
                    nc.vector.tensor_reduce(amax[:, j:j + 1], src, axis=AX.X,
                                            op=OP.max, apply_absolute_value=True)
                rs = scp.tile([P, 2], F32, tag="sc", name="rcp")
                nc.vector.reciprocal(rs[:], amax[:])
                s127 = scp.tile([P, 2], F32, tag="sc", name="s127")
                nc.vector.tensor_scalar_mul(s127[:], rs[:], 127.0)
                c = scp.tile([P, 2], F32, tag="sc", name="cc")
                nc.vector.tensor_scalar(c[:], amax[:], wsb[:, ws_idx:ws_idx + 1],
                                        None, OP.mult)
                return s127, c

            def round_dma_act(src_ap, s127_col, dst_slice):
                """magic-round on ACT (in-place), unmagic to bf16 on DVE,
                DMA-transpose into dst."""
                nc.scalar.activation(src_ap, src_ap, AF.Identity,
                                     scale=s127_col, bias=pmag[:])
                xq = xqsp.tile([P, DIM], BF16, tag="xqs", name="xqs")
                nc.vector.tensor_scalar(xq[:], src_ap, MAGIC, None, OP.subtract)
                nc.sync.dma_start_transpose(dst_slice, xq[:])

            def round_dma_dve(src_ap, s127_col, dst_slice):
                """magic-round + unmagic both on DVE (for attn phase where
                ACT is busy with exp)."""
                nc.vector.tensor_scalar(src_ap, src_ap, s127_col, MAGIC,
                                        OP.mult, OP.add)
                xq = xqsp.tile([P, DIM], BF16, tag="xqs", name="xqs")
                nc.vector.tensor_scalar(xq[:], src_ap, MAGIC, None, OP.subtract)
                nc.sync.dma_start_transpose(dst_slice, xq[:])

            # ---------------- patch embed ----------------
            posb_sb = wp.tile([P, 2, DIM], F32, tag="w", name="posb_sb")
            nc.sync.dma_start(posb_sb[:], posb_d.rearrange("(a p) d -> p a d", p=P))
            xpT_h = []
            for hf in range(2):
                xh = qkp.tile([P, KD, NTOK], F32, tag="qk", name="xpT")
                nc.sync.dma_start(
                    xh[:], xpT_d[:, hf * NTOK:(hf + 1) * NTOK].rearrange(
                        "(o p) t -> p o t", p=P))
                xpT_h.append(xh)
            pw_pieces = []
            for i in range(3):
                pwp = wp.tile([P, 2, DIM], F32, tag="w", name="pwp")
                nc.gpsimd.dma_start(
                    pwp[:], patchWT_d[i * 2 * P:(i + 1) * 2 * P, :].rearrange(
                        "(o p) d -> p o d", p=P))
                pw_pieces.append(pwp)
            for t in range(NT):
                for (cs, ce) in _mm_chunks(DIM):
                    pt = ps_mm.tile([P, 512], F32, tag="mm", name="pmm")[:, : ce - cs]
                    for k in range(KD):
                        nc.tensor.matmul(pt[:], xpT_h[t // 2][:, k, (t % 2) * P:(t % 2 + 1) * P],
                                         pw_pieces[k // 2][:, k % 2, cs:ce],
                                         start=(k == 0), stop=(k == KD - 1))
                    nc.vector.tensor_tensor(z[:, t, cs:ce], pt[:], posb_sb[:, t % 2, cs:ce], OP.add)

            def load_w(dram, b, kchunks, width, npieces):
                """Stage one linear's transposed fp8 weights as npieces tiles."""
                span = kchunks // npieces
                tiles = []
                for i in range(npieces):
                    wt = wp.tile([P, span, width], FP8, tag="w")
                    nc.gpsimd.dma_start(
                        wt[:],
                        dram[b, i * span * P:(i + 1) * span * P, :].rearrange(
                            "(o p) f -> p o f", p=P))
                    tiles.append(wt)
                return tiles, span

            def load_mods(b_, n_, name):
                mt = modp.tile([P, 2, 2, DIM], F32, tag="mod", name=name)
                nc.gpsimd.dma_start(mt[:], mods_d[b_, n_])
                return mt

            def p1_pair(pi, mt, ws_idx, half_dst, c_out, eng):
                """norm1/norm2 + quant for tile pair (2pi, 2pi+1); writes the
                transposed quantized half tile [P, KD, NTOK]."""
                ssq2 = scp.tile([P, 2], F32, tag="sc", name="ssq2")
                for j in range(2):
                    ssq_act(z[:, 2 * pi + j, :], ssq2[:, j:j + 1])
                rst2 = rstd_pair(ssq2[:])
                hs = []
                for j in range(2):
                    h = hp.tile([P, DIM], F32, tag="h")
                    norm_mod(2 * pi + j, mt, rst2[:, j:j + 1], h[:], eng)
                    hs.append(h)
                s127, cpr = quant_pair([h[:] for h in hs], ws_idx)
                for j in range(2):
                    c_out[2 * pi + j] = cpr[:, j:j + 1]
                    round_dma_act(hs[j][:], s127[:, j:j + 1],
                                  half_dst[:, :, j * P:(j + 1) * P])

            # ---- prologue: phase 1 of block 0 ----
            mt1_nxt = load_mods(0, 0, "mt1")
            xqT_cur = [xqtp.tile([P, KD, NTOK], BF16, tag="xqt", name="xqt0"), xqtp.tile([P, KD, NTOK], BF16, tag="xqt", name="xqt1")]
            c_cur = [None] * NT
            for pi in range(2):
                p1_pair(pi, mt1_nxt, 0, xqT_cur[pi], c_cur, nc.vector)

            for b in range(depth):
                xqT_h, c_list = xqT_cur, c_cur
                mt2 = load_mods(b, 1, "mt2")
                wq_tiles, wq_span = load_w(wqkv_d, b, KD, 3 * DIM, 3)

                with nc.named_scope(f"b{b}_qkv"):
                    q_fm = qkp.tile([P, KD, T], BF16, tag="qk", name="q_fm")
                    k_fm = qkp.tile([P, KD, T], BF16, tag="qk", name="k_fm")
                    cb = cbp.tile([P, T], F32, tag="cb", name="cbb")

                    def build_cb(half):
                        # per-token scales -> [1,256] row via PE transpose,
                        # then broadcast to all partitions.
                        crow_ps = ps_x.tile([1, 2, P], F32, tag="x", name="crow_ps")
                        for j in range(2):
                            nc.tensor.transpose(crow_ps[:, j, :],
                                                c_list[2 * half + j], idf[:])
                        crow = cbp.tile([1, NTOK], F32, tag="cb", name="crow")
                        nc.vector.tensor_copy(
                            crow[:], crow_ps[:].rearrange("a b c -> a (b c)"))
                        nc.gpsimd.partition_broadcast(
                            cb[:, half * NTOK:(half + 1) * NTOK], crow[0:1, :])

                    def v_mm(t):
                        for (cs, ce) in _mm_chunks(DIM):
                            pt = ps_mm.tile([P, 512], F32, tag="mm", name="pmm")[:, : ce - cs]
                            for k in range(KD):
                                wt = wq_tiles[k // wq_span]
                                nc.tensor.matmul(
                                    pt[:], xqT_h[t // 2][:, k, (t % 2) * P:(t % 2 + 1) * P],
                                    wt[:, k % wq_span, 2 * DIM + cs:2 * DIM + ce],
                                    start=(k == 0), stop=(k == KD - 1))
                            nc.scalar.activation(
                                v_aug[:, t, cs // HD:ce // HD, 0:HD], pt[:],
                                AF.Identity, scale=c_list[t])

                    def qk_mm(half):
                        # weight-stationary, psum [feat 128, tok 256]
                        ts0 = half * NTOK
                        for fc in range(12):
                            pt = ps_mm.tile([P, 512], F32, tag="mm",
                                            name="pmm")[:, :NTOK]
                            for k in range(KD):
                                wt = wq_tiles[k // wq_span]
                                nc.tensor.matmul(
                                    pt[:], wt[:, k % wq_span, fc * P:(fc + 1) * P],
                                    xqT_h[half][:, k, :],
                                    start=(k == 0), stop=(k == KD - 1))
                            if fc < 6:
                                nc.vector.scalar_tensor_tensor(
                                    q_fm[:, fc, ts0:ts0 + NTOK], pt[:], 0.125,
                                    cb[:, ts0:ts0 + NTOK], OP.mult, OP.mult)
                            else:
                                nc.vector.tensor_tensor(
                                    k_fm[:, fc - 6, ts0:ts0 + NTOK], pt[:],
                                    cb[:, ts0:ts0 + NTOK], OP.mult)

                    build_cb(0)
                    v_mm(0)
                    v_mm(1)
                    qk_mm(0)
                    build_cb(1)
                    v_mm(2)
                    v_mm(3)
                    qk_mm(1)

                # --- attention ---
                wp_tiles, wp_span = load_w(wproj_d, b, KD, DIM, 2)
                xqoT_h = [xqop.tile([P, KD, NTOK], BF16, tag="xqo",
                                    name=f"xqo{i}") for i in range(2)]
                co_s = [None] * NT

                def o_quant_pair(img):
                    srcs = [o_tm[:, 2 * img + j, :] for j in range(2)]
                    s127, cpr = quant_pair(srcs, 4 * b + 1)
                    for j in range(2):
                        co_s[2 * img + j] = cpr[:, j:j + 1]
                        round_dma_dve(srcs[j], s127[:, j:j + 1],
                                      xqoT_h[img][:, :, j * P:(j + 1) * P])

                with nc.named_scope(f"b{b}_attn"):
                    for img in range(2):
                        for g in range(2):
                            eTg = eTp.tile([P, 6, 2, NTOK], BF16, tag="eT")
                            for h6 in range(6):
                                hh = 6 * g + h6
                                po = (hh % 2) * HD
                                ch = hh // 2
                                lt = ps_lt.tile([P, 2, NTOK], F32, tag="lt")
                                for mt in range(2):
                                    nc.tensor.matmul(
                                        lt[:, mt, :],
                                        k_fm[po:po + HD, ch,
                                             img * NTOK + mt * P: img * NTOK + (mt + 1) * P],
                                        q_fm[po:po + HD, ch,
                                             img * NTOK: (img + 1) * NTOK],
                                        start=True, stop=True)
                                nc.scalar.activation(eTg[:, h6], lt[:], AF.Exp)
                            for nt in range(2):
                                oa = ps_oa.tile([P, 6, HD + 1], F32, tag="oa")
                                for h6 in range(6):
                                    for mt in range(2):
                                        nc.tensor.matmul(
                                            oa[:, h6, :],
                                            eTg[:, h6, mt, nt * P:(nt + 1) * P],
                                            v_aug[:, img * 2 + mt, 6 * g + h6, :],
                                            start=(mt == 0), stop=(mt == 1))
                                rinv = scp.tile([P, 6], F32, tag="sc", name="rinv")
                                nc.vector.reciprocal(rinv[:], oa[:, :, HD])
                                dst = o_tm[:, img * 2 + nt,
                                           384 * g:384 * (g + 1)].rearrange(
                                               "p (h d) -> p h d", h=6)
                                nc.vector.tensor_tensor(
                                    dst, oa[:, :, 0:HD],
                                    rinv[:, :, None].broadcast_to([P, 6, HD]),
                                    OP.mult)
                            if g == 1:
                                o_quant_pair(img)

                # --- proj + n2 chain ---
                wf1_tiles, wf1_span = load_w(wfc1_d, b, KD, HID, 3)
                xq2T_h = [xq2p.tile([P, KD, NTOK], BF16, tag="xq2",
                                    name=f"xq2{i}") for i in range(2)]
                c3s = [None] * NT

                with nc.named_scope(f"b{b}_proj"):
                    for t in range(NT):
                        for (cs, ce) in _mm_chunks(DIM):
                            pt = ps_mm.tile([P, 512], F32, tag="mm", name="pmm")[:, : ce - cs]
                            for k in range(KD):
                                wt = wp_tiles[k // wp_span]
                                nc.tensor.matmul(
                                    pt[:], xqoT_h[t // 2][:, k, (t % 2) * P:(t % 2 + 1) * P],
                                    wt[:, k % wp_span, cs:ce],
                                    start=(k == 0), stop=(k == KD - 1))
                            nc.vector.scalar_tensor_tensor(
                                z[:, t, cs:ce], pt[:], co_s[t], z[:, t, cs:ce],
                                OP.mult, OP.add)
                        if t % 2 == 1:
                            p1_pair(t // 2, mt2, 4 * b + 2, xq2T_h[t // 2], c3s,
                                    nc.vector)

                # --- fc1 + gelu + g-quant ---
                wf2_tiles, wf2_span = load_w(wfc2_d, b, KH, DIM, 3)
                xqg = [None] * NT
                c4s = [None] * NT
                gs = [None] * NT

                def gquant(t):
                    gh0, gh1 = gs[t]
                    am = scp.tile([P, 2], F32, tag="sc", name="amg")
                    nc.vector.tensor_reduce(am[:, 0:1], gh0[:], axis=AX.X, op=OP.max,
                                            apply_absolute_value=True)
                    nc.vector.tensor_reduce(am[:, 1:2], gh1[:], axis=AX.X, op=OP.max,
                                            apply_absolute_value=True)
                    ac = scp.tile([P, 1], F32, tag="sc", name="amaxc")
                    nc.vector.tensor_tensor(ac[:], am[:, 0:1], am[:, 1:2], OP.max)
                    rs = scp.tile([P, 1], F32, tag="sc", name="rcp")
                    nc.vector.reciprocal(rs[:], ac[:])
                    s127 = scp.tile([P, 1], F32, tag="sc", name="s127")
                    nc.vector.tensor_scalar_mul(s127[:], rs[:], 127.0)
                    c = scp.tile([P, 1], F32, tag="sc", name="cc")
                    nc.vector.tensor_scalar(c[:], ac[:], wsb[:, 4 * b + 3:4 * b + 4],
                                            None, OP.mult)
                    c4s[t] = c
                    xg = xqgp.tile([P, KH, P], BF16, tag="xqg")
                    xqg[t] = xg
                    for i, gh in enumerate((gh0, gh1)):
                        # magic on ACT (Identity, no table switch), unmagic DVE
                        nc.scalar.activation(gh[:], gh[:], AF.Identity,
                                             scale=s127[:], bias=pmag[:])
                        xq = xqsp.tile([P, HID // 2], BF16, tag="xqs", name="xq24s")
                        nc.vector.tensor_scalar(xq[:], gh[:], MAGIC, None, OP.subtract)
                        nc.sync.dma_start_transpose(xg[:, i * 12:(i + 1) * 12, :], xq[:])

                # --- fc1/fc2 interleaved per tile, fc2 fused with next p1 ---
                fuse = b + 1 < depth
                if fuse:
                    mt1_nxt = load_mods(b + 1, 0, "mt1")
                    xqT_cur = [xqtp.tile([P, KD, NTOK], BF16, tag="xqt",
                                          name=f"xqt{i}") for i in range(2)]
                    c_cur = [None] * NT

                def fc1_t(t):
                    gh0 = gp.tile([P, HID // 2], F32, tag="g")
                    gh1 = gp.tile([P, HID // 2], F32, tag="g")
                    gs[t] = (gh0, gh1)
                    for ci, (cs, ce) in enumerate(_mm_chunks(HID)):
                        pt = ps_mm.tile([P, 512], F32, tag="mm", name="pmm")[:, : ce - cs]
                        for k in range(KD):
                            wt = wf1_tiles[k // wf1_span]
                            nc.tensor.matmul(
                                pt[:], xq2T_h[t // 2][:, k, (t % 2) * P:(t % 2 + 1) * P],
                                wt[:, k % wf1_span, cs:ce],
                                start=(k == 0), stop=(k == KD - 1))
                        gh = gh0 if ci < 3 else gh1
                        off = cs - (0 if ci < 3 else HID // 2)
                        nc.scalar.activation(gh[:, off:off + ce - cs], pt[:],
                                             AF.Gelu_apprx_tanh, scale=c3s[t][:])

                def fc2_t(t):
                    for (cs, ce) in _mm_chunks(DIM):
                        pt = ps_mm.tile([P, 512], F32, tag="mm", name="pmm")[:, : ce - cs]
                        for k in range(KH):
                            wt = wf2_tiles[k // wf2_span]
                            nc.tensor.matmul(pt[:], xqg[t][:, k, :],
                                             wt[:, k % wf2_span, cs:ce],
                                             start=(k == 0), stop=(k == KH - 1))
                        nc.vector.scalar_tensor_tensor(
                            z[:, t, cs:ce], pt[:], c4s[t][:], z[:, t, cs:ce],
                            OP.mult, OP.add)

                with nc.named_scope(f"b{b}_mlp"):
                    for t in range(NT):
                        fc1_t(t)
                        if t > 0:
                            gquant(t - 1)
                            fc2_t(t - 1)
                        if t == 2 and fuse:
                            p1_pair(0, mt1_nxt, 4 * (b + 1), xqT_cur[0], c_cur,
                                    nc.gpsimd)
                    gquant(NT - 1)
                    fc2_t(NT - 1)
                    if fuse:
                        p1_pair(1, mt1_nxt, 4 * (b + 1), xqT_cur[1], c_cur,
                                nc.gpsimd)

            # ---------------- final norm + head ----------------
            with nc.named_scope("head"):
                hw_pieces = []
                for i in range(3):
                    hwp = wp.tile([P, 2, DIM], F32, tag="w", name="hwp")
                    nc.gpsimd.dma_start(
                        hwp[:], headWT_d[i * 2 * P:(i + 1) * 2 * P, :].rearrange(
                            "(o p) d -> p o d", p=P))
                    hw_pieces.append(hwp)
                hbrow = tmp_.tile([1, DIM], F32, tag="tm", name="hbrow")
                nc.sync.dma_start(hbrow[:], headb_d[:])
                hbb = wp.tile([P, DIM], F32, tag="w", name="hbb")
                nc.gpsimd.partition_broadcast(hbb[:], hbrow[0:1, :])
                rst_cols = []
                for pi in range(2):
                    ssq2 = scp.tile([P, 2], F32, tag="sc", name="ssqh")
                    for j in range(2):
                        ssq_act(z[:, 2 * pi + j, :], ssq2[:, j:j + 1])
                    rst2 = rstd_pair(ssq2[:])
                    rst_cols += [rst2[:, 0:1], rst2[:, 1:2]]
                for t in range(NT):
                    zn = hp.tile([P, DIM], F32, tag="h")
                    nc.vector.tensor_scalar_mul(zn[:], z[:, t, :], rst_cols[t])
                    znT = hp.tile([P, DIM], F32, tag="h")
                    for g0 in range(0, KD, 4):
                        gn = min(4, KD - g0)
                        ptb = ps_lt.tile([P, 512], F32, tag="lt", name="ptb")[:, : gn * P]
                        for j in range(gn):
                            nc.tensor.transpose(ptb[:, j * P:(j + 1) * P],
                                                zn[:, (g0 + j) * P:(g0 + j + 1) * P], idf[:])
                        nc.vector.tensor_copy(znT[:, g0 * P:(g0 + gn) * P], ptb[:])
                    for (cs, ce) in _mm_chunks(DIM):
                        pt = ps_mm.tile([P, 512], F32, tag="mm", name="pmm")[:, : ce - cs]
                        for k in range(KD):
                            nc.tensor.matmul(pt[:], znT[:, k * P:(k + 1) * P],
                                             hw_pieces[k // 2][:, k % 2, cs:ce],
                                             start=(k == 0), stop=(k == KD - 1))
                        ot = tmp_.tile([P, DIM], F32, tag="tm", name="ot")[:, : ce - cs]
                        nc.vector.tensor_tensor(ot[:], pt[:], hbb[:, cs:ce], OP.add)
                        nc.sync.dma_start(out_d[t * P:(t + 1) * P, cs:ce], ot[:])

    nc.compile()
    return nc


# ---------------------------------------------------------------------------
# host-side numerics (numpy, fp32 — matches jax CPU within ~1e-7)

def _gelu_tanh(x):
    x = x.astype(np.float32)
    c = np.float32(math.sqrt(2.0 / math.pi))
    return np.float32(0.5) * x * (np.float32(1.0) +
                                  np.tanh(c * (x + np.float32(0.044715) * x * x * x)))


def _time_embedding(t, t_w1, t_b1, t_w2, t_b2):
    half = DIM // 2
    freqs = np.exp(-np.log(10000.0) * np.arange(half, dtype=np.float32) / (half - 1)).astype(np.float32)
    args = t[:, None].astype(np.float32) * freqs[None, :]
    emb = np.concatenate([np.sin(args), np.cos(args)], axis=-1).astype(np.float32)
    h = _gelu_tanh(emb @ t_w1.T + t_b1)
    return (h @ t_w2.T + t_b2).astype(np.float32)


def _quant_w(w):
    ws = np.float32(np.mean(np.abs(w), dtype=np.float64)) + np.float32(1e-5)
    wq = np.clip(np.round(w.astype(np.float32) / ws), -1.0, 1.0)
    return wq, ws


def _prepare(inputs):
    x = np.asarray(inputs["x"], np.float32)
    t = np.asarray(inputs["t"], np.float32)
    B = x.shape[0]
    n_cores = 8
    per = B // n_cores  # 2
    p = PATCH
    hh = IMG // p

    xp = x.reshape(B, CIN, hh, p, hh, p).transpose(0, 2, 4, 1, 3, 5).reshape(B, hh * hh, CIN * p * p)

    t_emb = _time_embedding(t, inputs["t_w1"], inputs["t_b1"], inputs["t_w2"], inputs["t_b2"])
    silu = (t_emb / (1.0 + np.exp(-t_emb))).astype(np.float32)

    depth = DEPTH
    mods = np.zeros((depth, 2, B, 2, DIM), np.float32)  # [blk, norm, img, A/B, D]
    wscl = np.zeros((4 * depth,), np.float32)
    wq_all, wp_all, wf1_all, wf2_all = [], [], [], []
    for b in range(depth):
        mod = silu @ np.asarray(inputs["blk_ada_w"][b], np.float32).T + np.asarray(
            inputs["blk_ada_b"][b], np.float32)
        sh1, sc1, sh2, sc2 = np.split(mod, 4, axis=-1)
        n1 = np.asarray(inputs["blk_norm1"][b], np.float32)
        n2 = np.asarray(inputs["blk_norm2"][b], np.float32)
        mods[b, 0, :, 0, :] = n1[None, :] * (1.0 + sc1)
        mods[b, 0, :, 1, :] = sh1
        mods[b, 1, :, 0, :] = n2[None, :] * (1.0 + sc2)
        mods[b, 1, :, 1, :] = sh2

        for j, (nm, lst) in enumerate([("blk_qkv", wq_all), ("blk_proj", wp_all),
                                       ("blk_fc1", wf1_all), ("blk_fc2", wf2_all)]):
            wq, ws = _quant_w(np.asarray(inputs[nm][b], np.float32))
            lst.append(np.ascontiguousarray(wq.T).astype(ml_dtypes.float8_e4m3))
            wscl[4 * b + j] = ws / np.float32(127.0)

    wqkv = np.stack(wq_all)
    wproj = np.stack(wp_all)
    wfc1 = np.stack(wf1_all)
    wfc2 = np.stack(wf2_all)

    posb = (np.asarray(inputs["pos_embed"][0], np.float32) +
            np.asarray(inputs["patch_b"], np.float32)[None, :]).astype(np.float32)
    patchWT = np.ascontiguousarray(np.asarray(inputs["patch_w"], np.float32).T)
    norm_w = np.asarray(inputs["norm_w"], np.float32)
    headWT = np.ascontiguousarray(np.asarray(inputs["head_w"], np.float32).T * norm_w[:, None])
    headb = np.asarray(inputs["head_b"], np.float32)[None, :]

    key = ("prog", depth)
    if key not in _CACHED:
        _CACHED[key] = build_program(depth)
    nc = _CACHED[key]

    in_maps = []
    for c in range(n_cores):
        imgs = slice(c * per, (c + 1) * per)
        xpT = np.ascontiguousarray(xp[imgs].reshape(per * hh * hh, CIN * p * p).T)
        in_maps.append(dict(
            xpT=xpT, posb=posb, patchWT=patchWT, headWT=headWT, headb=headb,
            wqkv=wqkv, wproj=wproj, wfc1=wfc1, wfc2=wfc2,
            mods=np.ascontiguousarray(
                np.broadcast_to(mods[:, :, None, imgs], (depth, 2, 128, per, 2, DIM))),
            wscl=wscl[None, :],
        ))

    return nc, in_maps


def _assemble(res, B=16, per=2):
    p = PATCH
    hh = IMG // p
    out = np.zeros((B, CIN, IMG, IMG), np.float32)
    for c in range(B // per):
        zo = res.results[c]["zout"]  # [512, 768]
        for i in range(per):
            zi = zo[i * 256:(i + 1) * 256]
            out[c * per + i] = zi.reshape(hh, hh, CIN, p, p).transpose(2, 0, 3, 1, 4).reshape(CIN, IMG, IMG)
    return out


def kernel(**inputs):
    nc, in_maps = _prepare(inputs)
    res = run_bass_kernel_spmd(nc, in_maps, list(range(len(in_maps))), trace=False)
    return _assemble(res)
